# revision 1
# baseline (speedup 1.0000x reference)
"""MHA (RoPE + causal softmax attention + out-proj) on 8 NeuronCores.

Sharding: DP4 x TP2. Core c: batch b = c % 4, head-group g = c // 4
(8 heads per core). Each core computes a transposed partial output
outT = (y_local @ w_o_slice^T)^T in [D, L]; host sums the two head-group
partials per batch and transposes back.

All matmuls bf16 x bf16 -> fp32 PSUM. Layout strategy:
  Phase A: qkv natural layout [L, comps] via out = xT_tile.T @ w_chunk;
           RoPE applied with strided free-dim APs straight out of PSUM;
           rotated q/k and v staged to DRAM scratch (bf16).
  Phase B: per head, q/k loaded back transposed ([comps, L]) via DMA xbar
           transpose (prefetched during phase A from a right-side pool);
           scores computed transposed (k on partitions) so the attn
           weights come out ready to be the moving operand of the attn@V
           matmul (no on-chip transposes). exp on ScalarE with the
           1/sqrt(HD) scale fused. Causal: fully-masked k-tiles skipped,
           diagonal tiles masked with 4 constant [128,512] 0/1 masks.
           Softmax denominator via all-ones [128,128] matmul accumulated
           in PSUM (M=128 so the reciprocal is already partition-
           broadcast when it lands).
  Phase C: out-proj outT[e, q] = sum_d w_oT[d, e] * yT[d, q].
"""

import numpy as np
import ml_dtypes

import concourse.bass as bass
import concourse.tile as tile
import concourse.mybir as mybir
from concourse import bacc
from concourse.bass_utils import run_bass_kernel_spmd

BF16 = ml_dtypes.bfloat16
F32 = mybir.dt.float32
BF = mybir.dt.bfloat16

B, L, D, H, HD = 4, 2048, 2048, 16, 128
NH = 8                      # heads per core
DL = NH * HD                # 1024 local head dims
ROPE_BASE = 10000.0
ALPHA = float(HD) ** -0.5

LT = L // 128               # 16 L-tiles
DT = D // 128               # 16 D(contract)-tiles
NCH = 6                     # qkv chunks of 512 comps: q03,k03,v03,q47,k47,v47
QC = L // 512               # 4 q-chunks of 512
KT = L // 128               # 16 k-tiles


def _chunk_kind(c):
    # chunk order: q(heads0-3), k(0-3), v(0-3), q(4-7), k(4-7), v(4-7)
    return ("q", "k", "v")[c % 3], c // 3


def build_program(phases="ABC", la=3, scb=4, ypb=2, psab=3, patb=2, paob=3):
    nc = bacc.Bacc("TRN2", target_bir_lowering=False, debug=False, num_devices=8)

    xT = nc.dram_tensor("xT", [D, L], BF, kind="ExternalInput").ap()
    wqkvT = nc.dram_tensor("wqkvT", [D, 3 * DL], BF, kind="ExternalInput").ap()
    woT = nc.dram_tensor("woT", [DL, L], BF, kind="ExternalInput").ap()
    chalf = nc.dram_tensor("chalf", [L, 256], BF, kind="ExternalInput").ap()
    shalf = nc.dram_tensor("shalf", [L, 256], BF, kind="ExternalInput").ap()
    masks = nc.dram_tensor("masks", [4 * 128, 512], BF, kind="ExternalInput").ap()
    outT = nc.dram_tensor("outT", [D, L], F32, kind="ExternalOutput").ap()

    # DRAM staging for rotated q/k (natural layout) and v
    qrot = nc.dram_tensor("qrot", [L, DL], BF, kind="Internal").ap()
    krot = nc.dram_tensor("krot", [L, DL], BF, kind="Internal").ap()
    vnat = nc.dram_tensor("vnat", [L, DL], BF, kind="Internal").ap()

    with tile.TileContext(nc) as tc:
        with tc.tile_pool(name="outer", bufs=1) as outer, \
             tc.tile_pool(name="pBqk", bufs=2, side="right") as pb, \
             tc.tile_pool(name="pBm", bufs=1, side="right") as pbm:
            # persistent: per-head attn outputs yT (rhs of phase C)
            yts = []
            for h in range(NH):
                yt = outer.tile([128, L], BF, name=f"yt{h}", tag=f"yt{h}")
                if "B" not in phases:
                    nc.vector.memset(yt, 0.0)
                yts.append(yt)
            ones128 = outer.tile([128, 128], BF, name="ones128", tag="oc")
            nc.vector.memset(ones128, 1.0)
            mts = []
            for m in range(4):
                mt = pbm.tile([128, 512], BF, name=f"mask{m}", tag=f"mask{m}")
                nc.sync.dma_start(out=mt, in_=masks[m * 128:(m + 1) * 128, :])
                mts.append(mt)

            # ---------------- Phase A: QKV + RoPE ----------------
            with tc.tile_pool(name="pA", bufs=1) as pa, \
                 tc.tile_pool(name="pAw", bufs=2) as paw, \
                 tc.tile_pool(name="pAt", bufs=patb) as pat, \
                 tc.tile_pool(name="pAo", bufs=paob) as pao, \
                 tc.tile_pool(name="psA", bufs=psab, space="PSUM") as psa:
                # resident xT tiles [128, L] per D-tile
                xts = []
                for d in range(DT):
                    xt = pa.tile([128, L], BF, name=f"xt{d}", tag=f"xt{d}")
                    nc.sync.dma_start(out=xt, in_=xT[d * 128:(d + 1) * 128, :])
                    xts.append(xt)
                c_sb = pa.tile([128, LT, 256], BF, name="c_sb", tag="c_sb")
                nc.sync.dma_start(
                    out=c_sb, in_=chalf.rearrange("(i p) g -> p i g", p=128))
                s_sb = pa.tile([128, LT, 256], BF, name="s_sb", tag="s_sb")
                nc.sync.dma_start(
                    out=s_sb, in_=shalf.rearrange("(i p) g -> p i g", p=128))

                for c in range(NCH if "A" in phases else 0):
                    kind, grp = _chunk_kind(c)
                    # weight chunk: tiles [128, 512] per D-tile
                    wch = paw.tile([128, DT, 512], BF, name="wch", tag="wch")
                    nc.sync.dma_start(
                        out=wch,
                        in_=wqkvT[:, c * 512:(c + 1) * 512].rearrange(
                            "(d p) e -> p d e", p=128))
                    for i in range(LT):
                        pnat = psa.tile([128, 512], F32, name="pnat", tag="pnat")
                        for d in range(DT):
                            nc.tensor.matmul(
                                pnat,
                                xts[d][:, i * 128:(i + 1) * 128],
                                wch[:, d, :],
                                start=(d == 0), stop=(d == DT - 1))
                        if kind == "v":
                            vo = pao.tile([128, 512], BF, name="vo", tag="ro")
                            nc.scalar.copy(out=vo, in_=pnat)
                            nc.sync.dma_start(
                                out=vnat[i * 128:(i + 1) * 128,
                                         grp * 512:(grp + 1) * 512],
                                in_=vo)
                        else:
                            x1 = pnat[:, 0::2]
                            x2 = pnat[:, 1::2]
                            ct = c_sb[:, i, :]
                            st = s_sb[:, i, :]
                            t1 = pat.tile([128, 256], F32, name="t1", tag="t1")
                            nc.vector.tensor_mul(t1, x1, ct)
                            t2 = pat.tile([128, 256], F32, name="t2", tag="t2")
                            nc.vector.tensor_mul(t2, x2, st)
                            t3 = pat.tile([128, 256], F32, name="t3", tag="t3")
                            nc.vector.tensor_mul(t3, x2, ct)
                            t4 = pat.tile([128, 256], F32, name="t4", tag="t4")
                            nc.vector.tensor_mul(t4, x1, st)
                            ro = pao.tile([128, 512], BF, name="ro", tag="ro")
                            nc.vector.tensor_sub(ro[:, 0::2], t1, t2)
                            nc.vector.tensor_add(ro[:, 1::2], t3, t4)
                            dst = qrot if kind == "q" else krot
                            nc.sync.dma_start(
                                out=dst[i * 128:(i + 1) * 128,
                                        grp * 512:(grp + 1) * 512],
                                in_=ro)

            # ---------------- Phase B: attention per head ----------------
            with tc.tile_pool(name="pBw", bufs=1) as pbw:
                # phase-C weights: loaded early in B (left side, after A frees)
                wos = []
                for dd in range(NH):
                    wo = pbw.tile([128, L], BF, name=f"wo{dd}", tag=f"wo{dd}")
                    nc.sync.dma_start(
                        out=wo, in_=woT[dd * 128:(dd + 1) * 128, :])
                    wos.append(wo)

                with tc.tile_pool(name="pBa", bufs=4) as pba, \
                     tc.tile_pool(name="pBr", bufs=2) as pbr, \
                     tc.tile_pool(name="psS", bufs=scb, space="PSUM") as pss, \
                     tc.tile_pool(name="psY", bufs=ypb, space="PSUM") as psy, \
                     tc.tile_pool(name="psD", bufs=2, space="PSUM") as psd:
                    for h in range(NH if "B" in phases else 0):
                        qt = pb.tile([128, L], BF, name="qt", tag="qt")
                        nc.sync.dma_start_transpose(
                            out=qt, in_=qrot[:, h * 128:(h + 1) * 128])
                        kt = pb.tile([128, L], BF, name="kt", tag="kt")
                        nc.sync.dma_start_transpose(
                            out=kt, in_=krot[:, h * 128:(h + 1) * 128])
                        vt = pb.tile([128, KT, 128], BF, name="vt", tag="vt")
                        nc.sync.dma_start(
                            out=vt,
                            in_=vnat[:, h * 128:(h + 1) * 128].rearrange(
                                "(j p) d -> p j d", p=128))

                        for qc in range(QC):
                            nkt = 4 * qc + 4
                            ypsum = psy.tile([128, 512], F32, name="ypsum", tag="yp")
                            dpsum = psd.tile([128, 512], F32, name="dpsum", tag="dp")
                            ats = {}

                            def emit_score(j, qc=qc, ats=ats):
                                sc = pss.tile([128, 512], F32, name="sc", tag="sc")
                                nc.tensor.matmul(
                                    sc, kt[:, j * 128:(j + 1) * 128],
                                    qt[:, qc * 512:(qc + 1) * 512],
                                    start=True, stop=True)
                                at = pba.tile([128, 512], BF, name="at", tag="at")
                                nc.scalar.activation(
                                    out=at, in_=sc,
                                    func=mybir.ActivationFunctionType.Exp,
                                    scale=ALPHA)
                                m = j - 4 * qc
                                if m >= 0:
                                    nc.vector.tensor_mul(at, at, mts[m])
                                ats[j] = at

                            LOOKAHEAD = la
                            for j in range(min(LOOKAHEAD, nkt)):
                                emit_score(j)
                            for j in range(nkt):
                                if j + LOOKAHEAD < nkt:
                                    emit_score(j + LOOKAHEAD)
                                at = ats.pop(j)
                                nc.tensor.matmul(
                                    ypsum, vt[:, j, :], at,
                                    start=(j == 0), stop=(j == nkt - 1))
                                nc.tensor.matmul(
                                    dpsum, ones128, at,
                                    start=(j == 0), stop=(j == nkt - 1))
                            rbs = pbr.tile([128, 512], BF, name="rbs", tag="rbs")
                            with nc.allow_low_precision("softmax recip bf16"):
                                nc.vector.reciprocal(out=rbs, in_=dpsum)
                            nc.vector.tensor_mul(
                                yts[h][:, qc * 512:(qc + 1) * 512], ypsum, rbs)

                # ---------------- Phase C: out-projection ----------------
                with tc.tile_pool(name="pCo", bufs=4) as pco, \
                     tc.tile_pool(name="psC", bufs=3, space="PSUM") as psc:
                    for e in range(DT if "C" in phases else 0):
                        for qc in range(QC):
                            op = psc.tile([128, 512], F32, name="op", tag="op")
                            for dd in range(NH):
                                nc.tensor.matmul(
                                    op,
                                    wos[dd][:, e * 128:(e + 1) * 128],
                                    yts[dd][:, qc * 512:(qc + 1) * 512],
                                    start=(dd == 0), stop=(dd == NH - 1))
                            ot = pco.tile([128, 512], F32, name="ot", tag="ot")
                            nc.scalar.copy(out=ot, in_=op)
                            nc.sync.dma_start(
                                out=outT[e * 128:(e + 1) * 128,
                                         qc * 512:(qc + 1) * 512],
                                in_=ot)
    nc.compile()
    return nc


_NC_CACHE = None


def _get_program():
    global _NC_CACHE
    if _NC_CACHE is None:
        _NC_CACHE = build_program()
    return _NC_CACHE


def _host_inputs(x, w_qkv, w_o):
    inv = 1.0 / (ROPE_BASE ** (np.arange(0, HD, 2, dtype=np.float64) / HD))
    ang = np.arange(L, dtype=np.float64)[:, None] * inv[None, :]
    chalf = np.tile(np.cos(ang), (1, 4)).astype(BF16)          # [L, 256]
    shalf = np.tile(np.sin(ang), (1, 4)).astype(BF16)
    p = np.arange(128)[:, None]
    f = np.arange(512)[None, :]
    masks = np.concatenate(
        [(128 * m + p <= f).astype(BF16) for m in range(4)], axis=0)  # [512,512]

    in_maps = []
    for c in range(8):
        b, g = c % 4, c // 4
        qr = w_qkv[g * DL:(g + 1) * DL]
        kr = w_qkv[D + g * DL:D + (g + 1) * DL]
        vr = w_qkv[2 * D + g * DL:2 * D + (g + 1) * DL]
        wqkvT = np.ascontiguousarray(
            np.concatenate([qr[:512], kr[:512], vr[:512],
                            qr[512:], kr[512:], vr[512:]], axis=0).T
        ).astype(BF16)
        in_maps.append({
            "xT": np.ascontiguousarray(x[b].T).astype(BF16),
            "wqkvT": wqkvT,
            "woT": np.ascontiguousarray(
                w_o[:, g * DL:(g + 1) * DL].T).astype(BF16),
            "chalf": chalf,
            "shalf": shalf,
            "masks": masks,
        })
    return in_maps


def kernel(x, w_qkv, w_o, _trace=False):
    x = np.asarray(x, dtype=np.float32)
    w_qkv = np.asarray(w_qkv, dtype=np.float32)
    w_o = np.asarray(w_o, dtype=np.float32)
    nc = _get_program()
    in_maps = _host_inputs(x, w_qkv, w_o)
    res = run_bass_kernel_spmd(nc, in_maps, core_ids=list(range(8)),
                               trace=_trace)
    kernel.last_result = res
    parts = [r["outT"] for r in res.results]
    out = np.empty((B, L, D), dtype=np.float32)
    for b in range(B):
        out[b] = (parts[b] + parts[b + 4]).T
    return out



# revision 11
# speedup vs baseline: 1.0514x; 1.0514x over previous
"""MHA (RoPE + causal softmax attention + out-proj) on 8 NeuronCores.

Sharding: DP4 x TP2. Core c: batch b = c % 4, head-group g = c // 4
(8 heads per core). Each core computes a transposed partial output
outT = (y_local @ w_o_slice^T)^T in [D, L] (bf16); host sums the two
head-group partials per batch and transposes back.

All matmuls bf16 x bf16 -> fp32 PSUM. Layout strategy (v2):
  Phase A: qkv natural layout [L, comps] via out = xT_tile.T @ w_chunk,
           d-outer accumulation over 6-tile L-groups so the PE starts as
           soon as the first (x, w) d-pair lands. q/k head-dims are
           pre-permuted host-side to [even|odd] halves so RoPE uses
           packed (unstrided) APs: one ScalarE copy PSUM->bf16, then 6
           DVE ops. Rotated q/k tiles are transposed on the PE
           (128x128 transpose-matmuls -> bf16 PSUM -> one DVE copy)
           into SBUF-resident qT/kT [hd, L] per head - no DRAM round
           trip, no DMA-xbar transposes. v is staged to DRAM (natural
           layout) via gpsimd-issued DMAs.
  Phase B: per head, scores computed transposed (k on partitions) from
           SBUF qT/kT; exp on ScalarE with the 1/sqrt(HD) scale fused.
           Causal: fully-masked k-tiles skipped, diagonal k-tiles
           truncated to the valid q-range (matmul cols 512/384/256/128),
           single [128,128] triangle mask. Softmax denominator via DVE
           bf16 accumulation of the exp tiles + ONE all-ones matmul per
           (head, q-chunk) instead of a ones-matmul per k-tile.
  Phase C: out-proj outT[e, q] = sum_d w_oT[d, e] * yT[d, q], bf16 out.
"""

import numpy as np
import ml_dtypes

import concourse.bass as bass
import concourse.tile as tile
import concourse.mybir as mybir
from concourse import bacc
from concourse.bass_utils import run_bass_kernel_spmd

BF16 = ml_dtypes.bfloat16
F32 = mybir.dt.float32
BF = mybir.dt.bfloat16

B, L, D, H, HD = 4, 2048, 2048, 16, 128
NH = 8                      # heads per core
DL = NH * HD                # 1024 local head dims
ROPE_BASE = 10000.0
ALPHA = float(HD) ** -0.5

LT = L // 128               # 16 L-tiles
DT = D // 128               # 16 D(contract)-tiles
NCH = 6                     # qkv chunks of 512 comps: q03,k03,v03,q47,k47,v47
QC = L // 512               # 4 q-chunks of 512
KT = L // 128               # 16 k-tiles
GROUPS = [(0, 6), (6, 12), (12, 16)]

EXP = mybir.ActivationFunctionType.Exp


def _chunk_kind(c):
    # chunk order: q(heads0-3), k(0-3), v(0-3), q(4-7), k(4-7), v(4-7)
    return ("q", "k", "v")[c % 3], c // 3


def build_program(phases="ABC", la=1):
    nc = bacc.Bacc("TRN2", target_bir_lowering=False, debug=False, num_devices=8)

    xT = nc.dram_tensor("xT", [D, L], BF, kind="ExternalInput").ap()
    wqkvT = nc.dram_tensor("wqkvT", [D, 3 * DL], BF, kind="ExternalInput").ap()
    woT = nc.dram_tensor("woT", [DL, D], BF, kind="ExternalInput").ap()
    chalf = nc.dram_tensor("chalf", [L, 256], BF, kind="ExternalInput").ap()
    shalf = nc.dram_tensor("shalf", [L, 256], BF, kind="ExternalInput").ap()
    mtri_d = nc.dram_tensor("mtri", [128, 128], BF, kind="ExternalInput").ap()
    ident_d = nc.dram_tensor("ident", [128, 128], BF, kind="ExternalInput").ap()
    outT = nc.dram_tensor("outT", [D, L], BF, kind="ExternalOutput").ap()

    # DRAM staging for v (natural layout), split per head-group so the
    # vt reloads for heads 0-3 have no dependency on the group-1 writes
    vnat0 = nc.dram_tensor("vnat0", [L, 512], BF, kind="Internal").ap()
    vnat1 = nc.dram_tensor("vnat1", [L, 512], BF, kind="Internal").ap()

    with tile.TileContext(nc) as tc:
        with tc.tile_pool(name="outer", bufs=1) as outer:
            # persistent: transposed rotated q/k per head [hd, L]
            qT = outer.tile([128, NH, L], BF, name="qT", tag="qT")
            kT = outer.tile([128, NH, L], BF, name="kT", tag="kT")
            c_sb = outer.tile([128, LT, 4, 64], BF, name="c_sb", tag="c_sb")
            s_sb = outer.tile([128, LT, 4, 64], BF, name="s_sb", tag="s_sb")
            ones128 = outer.tile([128, 128], BF, name="ones128", tag="oc")
            zeros = outer.tile([128, 512], BF, name="zeros", tag="zc")
            mtri = outer.tile([128, 128], BF, name="mtri", tag="mtri")
            idt = outer.tile([128, 128], BF, name="idt", tag="idt")
            nc.vector.memset(ones128, 1.0)
            nc.vector.memset(zeros, 0.0)
            if "B" not in phases:
                nc.vector.memset(qT, 0.0)
                nc.vector.memset(kT, 0.0)

            # small phase-B pools opened FIRST so their SBUF addresses do
            # not overlap the phase-A zone (no release dependency on the
            # phase-A drain)
            pbv01 = tc.alloc_tile_pool(name="pBv01", bufs=1)
            pba = tc.alloc_tile_pool(name="pBa", bufs=4)
            pac = tc.alloc_tile_pool(name="pBc", bufs=2)
            pbr = tc.alloc_tile_pool(name="pBr", bufs=2)
            vts = [None] * 4

            # ---------------- Phase A: QKV + RoPE + transpose ----------------
            with tc.tile_pool(name="pA", bufs=1) as pa, \
                 tc.tile_pool(name="pAw", bufs=2) as paw, \
                 tc.tile_pool(name="pAt", bufs=2) as pat, \
                 tc.tile_pool(name="pAo", bufs=3) as pao, \
                 tc.tile_pool(name="pAr", bufs=2) as pro, \
                 tc.tile_pool(name="psA", bufs=1, space="PSUM") as psa, \
                 tc.tile_pool(name="psT", bufs=2, space="PSUM") as pst:
                # PE warm-up: a few dummy matmuls so the p-state ramp runs
                # during the initial DMA loads instead of on real work
                warm = psa.tile([128, 512], F32, name="pn", tag="pn0")
                for _ in range(8):
                    nc.tensor.matmul(warm, ones128, zeros, start=True,
                                     stop=True)
                # resident xT tiles [128, L] per D-tile; loaded in column
                # halves interleaved with chunk-0 weight d-tiles so the
                # d-outer accumulation can start almost immediately.
                xts = [pa.tile([128, L], BF, name=f"xt{d}", tag=f"xt{d}")
                       for d in range(DT)]
                wch0 = paw.tile([128, DT, 512], BF, name="wch", tag="wch")
                for d in range(DT):
                    nc.sync.dma_start(out=xts[d][:, 0:1024],
                                      in_=xT[d * 128:(d + 1) * 128, 0:1024])
                    nc.sync.dma_start(
                        out=wch0[:, d, :],
                        in_=wqkvT[d * 128:(d + 1) * 128, 0:512])
                nc.sync.dma_start(
                    out=c_sb,
                    in_=chalf.rearrange("(i p) g -> p i g", p=128))
                nc.sync.dma_start(
                    out=s_sb,
                    in_=shalf.rearrange("(i p) g -> p i g", p=128))
                nc.sync.dma_start(out=idt, in_=ident_d)
                nc.sync.dma_start(out=mtri, in_=mtri_d)
                for d in range(DT):
                    nc.sync.dma_start(out=xts[d][:, 1024:2048],
                                      in_=xT[d * 128:(d + 1) * 128, 1024:2048])

                for c in range(NCH if "A" in phases else 0):
                    kind, grp = _chunk_kind(c)
                    if c == 0:
                        wch = wch0
                    else:
                        wch = paw.tile([128, DT, 512], BF, name="wch", tag="wch")
                        for d in range(DT):
                            nc.sync.dma_start(
                                out=wch[:, d, :],
                                in_=wqkvT[d * 128:(d + 1) * 128,
                                          c * 512:(c + 1) * 512])
                    for (lo, hi) in GROUPS:
                        pns = [psa.tile([128, 512], F32, name="pn", tag=f"pn{i - lo}")
                               for i in range(lo, hi)]
                        for d in range(DT):
                            for i in range(lo, hi):
                                nc.tensor.matmul(
                                    pns[i - lo],
                                    xts[d][:, i * 128:(i + 1) * 128],
                                    wch[:, d, :],
                                    start=(d == 0), stop=(d == DT - 1))
                        for i in range(lo, hi):
                            pn = pns[i - lo]
                            if kind == "v":
                                vo = pao.tile([128, 512], BF, name="vo", tag="vo")
                                nc.scalar.copy(out=vo, in_=pn)
                                vn = vnat0 if grp == 0 else vnat1
                                nc.gpsimd.dma_start(
                                    out=vn[i * 128:(i + 1) * 128, :],
                                    in_=vo)
                            else:
                                xb = pao.tile([128, 4, 128], BF, name="xb", tag="vo")
                                nc.scalar.copy(out=xb, in_=pn)
                                x1 = xb[:, :, 0:64]
                                x2 = xb[:, :, 64:128]
                                ct = c_sb[:, i, :, :]
                                st = s_sb[:, i, :, :]
                                t1 = pat.tile([128, 4, 64], BF, name="t1", tag="t1")
                                nc.vector.tensor_mul(t1, x1, ct)
                                t2 = pat.tile([128, 4, 64], BF, name="t2", tag="t2")
                                nc.vector.tensor_mul(t2, x2, st)
                                t3 = pat.tile([128, 4, 64], BF, name="t3", tag="t3")
                                nc.vector.tensor_mul(t3, x2, ct)
                                t4 = pat.tile([128, 4, 64], BF, name="t4", tag="t4")
                                nc.vector.tensor_mul(t4, x1, st)
                                ro = pro.tile([128, 4, 128], BF, name="ro", tag="ro")
                                nc.vector.tensor_sub(ro[:, :, 0:64], t1, t2)
                                nc.vector.tensor_add(ro[:, :, 64:128], t3, t4)
                                pt = pst.tile([128, 4, 128], BF, name="pt", tag="pt")
                                for hh in range(4):
                                    nc.tensor.transpose(
                                        pt[:, hh, :], ro[:, hh, :], idt)
                                dstT = qT if kind == "q" else kT
                                nc.vector.tensor_copy(
                                    out=dstT[:, grp * 4:(grp + 1) * 4,
                                             i * 128:(i + 1) * 128],
                                    in_=pt)
                    if kind == "v" and grp == 0 and "B" in phases:
                        # v for heads 0-3 fully staged: prefetch vt pair 0
                        vt = pbv01.tile([128, KT, 256], BF,
                                        name="vt0", tag="vt0")
                        nc.sync.dma_start(
                            out=vt,
                            in_=vnat0[:, 0:256].rearrange(
                                "(j p) d -> p j d", p=128))
                        vts[0] = vt

            # ---------------- Phase B+C: attention (qc-outer) with the
            # out-projection of the previous q-chunk interleaved into the
            # PE stream as filler for the exp-bound stretches ----------------
            with tc.tile_pool(name="pBw", bufs=1) as pbw, \
                 tc.tile_pool(name="pBv23", bufs=1) as pbv23, \
                 tc.tile_pool(name="pBy", bufs=2) as pby, \
                 tc.tile_pool(name="pCo", bufs=4) as pco:
                for hp in (1, 2, 3):
                    vn, c0 = (vnat0, 256) if hp == 1 else (vnat1, (hp - 2) * 256)
                    vt = pbv23.tile([128, KT, 256], BF, name=f"vt{hp}",
                                    tag=f"vt{hp}")
                    nc.sync.dma_start(
                        out=vt,
                        in_=vn[:, c0:c0 + 256].rearrange(
                            "(j p) d -> p j d", p=128))
                    vts[hp] = vt
                # phase-C weights: loaded early in B (after A frees)
                wos = []
                for dd in range(NH):
                    wo = pbw.tile([128, D], BF, name=f"wo{dd}", tag=f"wo{dd}")
                    nc.sync.dma_start(
                        out=wo, in_=woT[dd * 128:(dd + 1) * 128, :])
                    wos.append(wo)

                with tc.tile_pool(name="psS", bufs=2, space="PSUM") as pss, \
                     tc.tile_pool(name="psY", bufs=2, space="PSUM") as psy, \
                     tc.tile_pool(name="psC", bufs=2, space="PSUM") as psc:
                    # deferred per-(h, qc) epilogue so the denominator chain
                    # never blocks the PE FIFO
                    pending = [None]

                    def finalize():
                        if pending[0] is None:
                            return
                        ypsum_f, acc_f, h_f, yq_f, qc_f = pending[0]
                        pending[0] = None
                        if qc_f == 0:
                            # close the psum accumulation group on columns
                            # the truncated diagonal tiles never re-touched
                            nc.tensor.matmul(
                                ypsum_f[:, 0:384], ones128, zeros[:, 0:384],
                                start=False, stop=True)
                        dps = pss.tile([128, 2, 512], F32, name="dps", tag="sc")
                        nc.tensor.matmul(dps[:, 0, :], ones128, acc_f,
                                         start=True, stop=True)
                        rbs = pbr.tile([128, 512], BF, name="rbs", tag="rbs")
                        with nc.allow_low_precision("softmax recip bf16"):
                            nc.vector.reciprocal(out=rbs, in_=dps[:, 0, :])
                        nc.vector.tensor_mul(yq_f[:, h_f, :], ypsum_f, rbs)

                    def c_gen(yq, qcc):
                        # out-projection of q-chunk qcc, one PE matmul per
                        # yield step
                        for e in range(DT):
                            op = psc.tile([128, 512], F32, name="op", tag="op")
                            for dd in range(NH):
                                nc.tensor.matmul(
                                    op,
                                    wos[dd][:, e * 128:(e + 1) * 128],
                                    yq[:, dd, :],
                                    start=(dd == 0), stop=(dd == NH - 1))
                                yield
                            ot = pco.tile([128, 512], BF, name="ot", tag="ot")
                            nc.gpsimd.tensor_copy(ot, op)
                            nc.sync.dma_start(
                                out=outT[e * 128:(e + 1) * 128,
                                         qcc * 512:(qcc + 1) * 512],
                                in_=ot)
                            yield

                    cg = [None]

                    def c_step(k):
                        if cg[0] is None:
                            return
                        for _ in range(k):
                            if next(cg[0], "done") == "done":
                                cg[0] = None
                                return

                    for qc in range(QC if "B" in phases else 0):
                        yq = pby.tile([128, NH, 512], BF, name="yq", tag="yq")
                        for h in range(NH):
                            hp, h2 = h // 2, h % 2
                            vt = vts[hp]
                            # entries: diagonal k-tiles as singles (truncated
                            # widths), below-diagonal k-tiles paired
                            # two-per-exp to amortize ScalarE access latency
                            entries = [[(4 * qc + m, 128 * m)]
                                       for m in range(4)]
                            entries += [[(j, 0), (j + 1, 0)]
                                        for j in range(0, 4 * qc, 2)]
                            n = len(entries)
                            ypsum = psy.tile([128, 512], F32, name="ypsum",
                                             tag="yp")
                            acc = pac.tile([128, 512], BF, name="acc",
                                           tag="acc")
                            ats = {}

                            def emit(ei, h=h, qc=qc, entries=entries,
                                     acc=acc, ats=ats):
                                slots = entries[ei]
                                sc = pss.tile([128, 2, 512], F32,
                                              name="sc", tag="sc")
                                for s, (jg, c0) in enumerate(slots):
                                    nc.tensor.matmul(
                                        sc[:, s, c0:512],
                                        kT[:, h, jg * 128:(jg + 1) * 128],
                                        qT[:, h,
                                           qc * 512 + c0:(qc + 1) * 512],
                                        start=True, stop=True)
                                at = pba.tile([128, 2, 512], BF,
                                              name="at", tag="at")
                                if len(slots) == 2:
                                    nc.scalar.activation(
                                        out=at, in_=sc, func=EXP,
                                        scale=ALPHA)
                                else:
                                    jg, c0 = slots[0]
                                    nc.scalar.activation(
                                        out=at[:, 0, c0:512],
                                        in_=sc[:, 0, c0:512],
                                        func=EXP, scale=ALPHA)
                                    nc.vector.tensor_mul(
                                        at[:, 0, c0:c0 + 128],
                                        at[:, 0, c0:c0 + 128], mtri)
                                if ei == 0:
                                    nc.vector.tensor_copy(out=acc,
                                                          in_=at[:, 0, :])
                                else:
                                    for s, (jg, c0) in enumerate(slots):
                                        nc.vector.tensor_add(
                                            acc[:, c0:512],
                                            acc[:, c0:512],
                                            at[:, s, c0:512])
                                ats[ei] = (at, slots)

                            for ei in range(min(la, n)):
                                emit(ei)
                            finalize()
                            for ei in range(n):
                                if ei + la < n:
                                    emit(ei + la)
                                at, slots = ats.pop(ei)
                                for s, (jg, c0) in enumerate(slots):
                                    nc.tensor.matmul(
                                        ypsum[:, c0:512],
                                        vt[:, jg,
                                           h2 * 128:(h2 + 1) * 128],
                                        at[:, s, c0:512],
                                        start=(ei == 0 and s == 0),
                                        stop=(ei == n - 1 and
                                              s == len(slots) - 1))
                                c_step(3)
                            pending[0] = (ypsum, acc, h, yq, qc)
                        finalize()
                        c_step(200)  # drain any leftover out-proj steps
                        cg[0] = c_gen(yq, qc)
                    c_step(200)

            pbr.release()
            pac.release()
            pba.release()
            pbv01.release()
    nc.compile()
    return nc


_NC_CACHE = None


def _get_program():
    global _NC_CACHE
    if _NC_CACHE is None:
        _NC_CACHE = build_program()
    return _NC_CACHE


def _host_inputs(x, w_qkv, w_o):
    inv = 1.0 / (ROPE_BASE ** (np.arange(0, HD, 2, dtype=np.float64) / HD))
    ang = np.arange(L, dtype=np.float64)[:, None] * inv[None, :]
    chalf = np.tile(np.cos(ang), (1, 4)).astype(BF16)          # [L, 256]
    shalf = np.tile(np.sin(ang), (1, 4)).astype(BF16)
    p = np.arange(128)[:, None]
    f = np.arange(128)[None, :]
    mtri = (p <= f).astype(BF16)                               # [128, 128]
    ident = np.eye(128).astype(BF16)
    # per-head [even dims | odd dims] permutation for q/k rows
    perm = np.concatenate([np.arange(0, 128, 2), np.arange(1, 128, 2)])

    in_maps = []
    for c in range(8):
        b, g = c % 4, c // 4
        qr = w_qkv[g * DL:(g + 1) * DL]
        kr = w_qkv[D + g * DL:D + (g + 1) * DL]
        vr = w_qkv[2 * D + g * DL:2 * D + (g + 1) * DL]
        qp = qr.reshape(NH, HD, D)[:, perm, :].reshape(DL, D)
        kp = kr.reshape(NH, HD, D)[:, perm, :].reshape(DL, D)
        wqkvT = np.ascontiguousarray(
            np.concatenate([qp[:512], kp[:512], vr[:512],
                            qp[512:], kp[512:], vr[512:]], axis=0).T
        ).astype(BF16)
        in_maps.append({
            "xT": np.ascontiguousarray(x[b].T).astype(BF16),
            "wqkvT": wqkvT,
            "woT": np.ascontiguousarray(
                w_o[:, g * DL:(g + 1) * DL].T).astype(BF16),
            "chalf": chalf,
            "shalf": shalf,
            "mtri": mtri,
            "ident": ident,
        })
    return in_maps


def kernel(x, w_qkv, w_o, _trace=False):
    x = np.asarray(x, dtype=np.float32)
    w_qkv = np.asarray(w_qkv, dtype=np.float32)
    w_o = np.asarray(w_o, dtype=np.float32)
    nc = _get_program()
    in_maps = _host_inputs(x, w_qkv, w_o)
    res = run_bass_kernel_spmd(nc, in_maps, core_ids=list(range(8)),
                               trace=_trace)
    kernel.last_result = res
    parts = [np.asarray(r["outT"], dtype=np.float32) for r in res.results]
    out = np.empty((B, L, D), dtype=np.float32)
    for b in range(B):
        out[b] = (parts[b] + parts[b + 4]).T
    return out


# revision 12
# speedup vs baseline: 1.1021x; 1.0483x over previous
"""MHA (RoPE + causal softmax attention + out-proj) on 8 NeuronCores.

Sharding: DP4 x TP2. Core c: batch b = c % 4, head-group g = c // 4
(8 heads per core). Each core computes a transposed partial output
outT = (y_local @ w_o_slice^T)^T in [D, L] (bf16); host sums the two
head-group partials per batch and transposes back.

All matmuls bf16 x bf16 -> fp32 PSUM. Layout strategy (v2):
  Phase A: qkv natural layout [L, comps] via out = xT_tile.T @ w_chunk,
           d-outer accumulation over 6-tile L-groups so the PE starts as
           soon as the first (x, w) d-pair lands. q/k head-dims are
           pre-permuted host-side to [even|odd] halves so RoPE uses
           packed (unstrided) APs: one ScalarE copy PSUM->bf16, then 6
           DVE ops. Rotated q/k tiles are transposed on the PE
           (128x128 transpose-matmuls -> bf16 PSUM -> one DVE copy)
           into SBUF-resident qT/kT [hd, L] per head - no DRAM round
           trip, no DMA-xbar transposes. v is staged to DRAM (natural
           layout) via gpsimd-issued DMAs.
  Phase B: per head, scores computed transposed (k on partitions) from
           SBUF qT/kT; exp on ScalarE with the 1/sqrt(HD) scale fused.
           Causal: fully-masked k-tiles skipped, diagonal k-tiles
           truncated to the valid q-range (matmul cols 512/384/256/128),
           single [128,128] triangle mask. Softmax denominator via DVE
           bf16 accumulation of the exp tiles + ONE all-ones matmul per
           (head, q-chunk) instead of a ones-matmul per k-tile.
  Phase C: out-proj outT[e, q] = sum_d w_oT[d, e] * yT[d, q], bf16 out.
"""

import numpy as np
import ml_dtypes

import concourse.bass as bass
import concourse.tile as tile
import concourse.mybir as mybir
from concourse import bacc
from concourse.bass_utils import run_bass_kernel_spmd

BF16 = ml_dtypes.bfloat16
F32 = mybir.dt.float32
BF = mybir.dt.bfloat16

B, L, D, H, HD = 4, 2048, 2048, 16, 128
NH = 8                      # heads per core
DL = NH * HD                # 1024 local head dims
ROPE_BASE = 10000.0
ALPHA = float(HD) ** -0.5

LT = L // 128               # 16 L-tiles
DT = D // 128               # 16 D(contract)-tiles
NCH = 6                     # qkv chunks of 512 comps: q03,k03,v03,q47,k47,v47
QC = L // 512               # 4 q-chunks of 512
KT = L // 128               # 16 k-tiles
GROUPS = [(0, 6), (6, 12), (12, 16)]

EXP = mybir.ActivationFunctionType.Exp


def _chunk_kind(c):
    # chunk order: q(heads0-3), k(0-3), v(0-3), q(4-7), k(4-7), v(4-7)
    return ("q", "k", "v")[c % 3], c // 3


def build_program(phases="ABC", la=1):
    nc = bacc.Bacc("TRN2", target_bir_lowering=False, debug=False, num_devices=8)

    xT = nc.dram_tensor("xT", [D, L], BF, kind="ExternalInput").ap()
    wqkvT = nc.dram_tensor("wqkvT", [D, 3 * DL], BF, kind="ExternalInput").ap()
    woT = nc.dram_tensor("woT", [DL, D], BF, kind="ExternalInput").ap()
    chalf = nc.dram_tensor("chalf", [L, 256], BF, kind="ExternalInput").ap()
    shalf = nc.dram_tensor("shalf", [L, 256], BF, kind="ExternalInput").ap()
    mtri_d = nc.dram_tensor("mtri", [128, 128], BF, kind="ExternalInput").ap()
    ident_d = nc.dram_tensor("ident", [128, 128], BF, kind="ExternalInput").ap()
    outT = nc.dram_tensor("outT", [D, L], BF, kind="ExternalOutput").ap()

    # DRAM staging for v (natural layout), split per head-group so the
    # vt reloads for heads 0-3 have no dependency on the group-1 writes
    vnat0 = nc.dram_tensor("vnat0", [L, 512], BF, kind="Internal").ap()
    vnat1 = nc.dram_tensor("vnat1", [L, 512], BF, kind="Internal").ap()

    with tile.TileContext(nc) as tc:
        with tc.tile_pool(name="outer", bufs=1) as outer:
            # persistent: transposed rotated q/k per head [hd, L]
            qT = outer.tile([128, NH, L], BF, name="qT", tag="qT")
            kT = outer.tile([128, NH, L], BF, name="kT", tag="kT")
            c_sb = outer.tile([128, LT, 4, 64], BF, name="c_sb", tag="c_sb")
            s_sb = outer.tile([128, LT, 4, 64], BF, name="s_sb", tag="s_sb")
            ones128 = outer.tile([128, 128], BF, name="ones128", tag="oc")
            zeros = outer.tile([128, 512], BF, name="zeros", tag="zc")
            mtri = outer.tile([128, 128], BF, name="mtri", tag="mtri")
            idt = outer.tile([128, 128], BF, name="idt", tag="idt")
            nc.vector.memset(ones128, 1.0)
            nc.vector.memset(zeros, 0.0)
            if "B" not in phases:
                nc.vector.memset(qT, 0.0)
                nc.vector.memset(kT, 0.0)

            # small phase-B pools opened FIRST so their SBUF addresses do
            # not overlap the phase-A zone (no release dependency on the
            # phase-A drain)
            pbv01 = tc.alloc_tile_pool(name="pBv01", bufs=1)
            pba = tc.alloc_tile_pool(name="pBa", bufs=4)
            pac = tc.alloc_tile_pool(name="pBc", bufs=2)
            pbr = tc.alloc_tile_pool(name="pBr", bufs=2)
            vts = [None] * 4

            # ---------------- Phase A: QKV + RoPE + transpose ----------------
            with tc.tile_pool(name="pA", bufs=1) as pa, \
                 tc.tile_pool(name="pAw", bufs=2) as paw, \
                 tc.tile_pool(name="pAt", bufs=2) as pat, \
                 tc.tile_pool(name="pAo", bufs=3) as pao, \
                 tc.tile_pool(name="pAr", bufs=2) as pro, \
                 tc.tile_pool(name="psA", bufs=1, space="PSUM") as psa, \
                 tc.tile_pool(name="psT", bufs=2, space="PSUM") as pst:
                # PE warm-up: a few dummy matmuls so the p-state ramp runs
                # during the initial DMA loads instead of on real work
                warm = psa.tile([128, 512], F32, name="pn", tag="pn0")
                for _ in range(8):
                    nc.tensor.matmul(warm, ones128, zeros, start=True,
                                     stop=True)
                # resident xT tiles [128, L] per D-tile; loaded in column
                # halves interleaved with chunk-0 weight d-tiles so the
                # d-outer accumulation can start almost immediately.
                xts = [pa.tile([128, L], BF, name=f"xt{d}", tag=f"xt{d}")
                       for d in range(DT)]
                wch0 = paw.tile([128, DT, 512], BF, name="wch", tag="wch")
                for d in range(DT):
                    nc.sync.dma_start(out=xts[d][:, 0:1024],
                                      in_=xT[d * 128:(d + 1) * 128, 0:1024])
                    nc.sync.dma_start(
                        out=wch0[:, d, :],
                        in_=wqkvT[d * 128:(d + 1) * 128, 0:512])
                nc.sync.dma_start(
                    out=c_sb,
                    in_=chalf.rearrange("(i p) g -> p i g", p=128))
                nc.sync.dma_start(
                    out=s_sb,
                    in_=shalf.rearrange("(i p) g -> p i g", p=128))
                nc.sync.dma_start(out=idt, in_=ident_d)
                nc.sync.dma_start(out=mtri, in_=mtri_d)
                for d in range(DT):
                    nc.sync.dma_start(out=xts[d][:, 1024:2048],
                                      in_=xT[d * 128:(d + 1) * 128, 1024:2048])

                for c in range(NCH if "A" in phases else 0):
                    kind, grp = _chunk_kind(c)
                    if c == 0:
                        wch = wch0
                    else:
                        wch = paw.tile([128, DT, 512], BF, name="wch", tag="wch")
                        for d in range(DT):
                            nc.sync.dma_start(
                                out=wch[:, d, :],
                                in_=wqkvT[d * 128:(d + 1) * 128,
                                          c * 512:(c + 1) * 512])
                    for (lo, hi) in GROUPS:
                        pns = [psa.tile([128, 512], F32, name="pn", tag=f"pn{i - lo}")
                               for i in range(lo, hi)]
                        for d in range(DT):
                            for i in range(lo, hi):
                                nc.tensor.matmul(
                                    pns[i - lo],
                                    xts[d][:, i * 128:(i + 1) * 128],
                                    wch[:, d, :],
                                    start=(d == 0), stop=(d == DT - 1))
                        for i in range(lo, hi):
                            pn = pns[i - lo]
                            if kind == "v":
                                vo = pao.tile([128, 512], BF, name="vo", tag="vo")
                                nc.scalar.copy(out=vo, in_=pn)
                                vn = vnat0 if grp == 0 else vnat1
                                nc.gpsimd.dma_start(
                                    out=vn[i * 128:(i + 1) * 128, :],
                                    in_=vo)
                            else:
                                xb = pao.tile([128, 4, 128], BF, name="xb", tag="vo")
                                nc.scalar.copy(out=xb, in_=pn)
                                x1 = xb[:, :, 0:64]
                                x2 = xb[:, :, 64:128]
                                ct = c_sb[:, i, :, :]
                                st = s_sb[:, i, :, :]
                                t1 = pat.tile([128, 4, 64], BF, name="t1", tag="t1")
                                nc.vector.tensor_mul(t1, x1, ct)
                                t2 = pat.tile([128, 4, 64], BF, name="t2", tag="t2")
                                nc.vector.tensor_mul(t2, x2, st)
                                t3 = pat.tile([128, 4, 64], BF, name="t3", tag="t3")
                                nc.vector.tensor_mul(t3, x2, ct)
                                t4 = pat.tile([128, 4, 64], BF, name="t4", tag="t4")
                                nc.vector.tensor_mul(t4, x1, st)
                                ro = pro.tile([128, 4, 128], BF, name="ro", tag="ro")
                                nc.vector.tensor_sub(ro[:, :, 0:64], t1, t2)
                                nc.vector.tensor_add(ro[:, :, 64:128], t3, t4)
                                pt = pst.tile([128, 4, 128], BF, name="pt", tag="pt")
                                for hh in range(4):
                                    nc.tensor.transpose(
                                        pt[:, hh, :], ro[:, hh, :], idt)
                                dstT = qT if kind == "q" else kT
                                nc.vector.tensor_copy(
                                    out=dstT[:, grp * 4:(grp + 1) * 4,
                                             i * 128:(i + 1) * 128],
                                    in_=pt)
                    if kind == "v" and grp == 0 and "B" in phases:
                        # v for heads 0-3 fully staged: prefetch vt pair 0
                        vt = pbv01.tile([128, KT, 256], BF,
                                        name="vt0", tag="vt0")
                        nc.sync.dma_start(
                            out=vt,
                            in_=vnat0[:, 0:256].rearrange(
                                "(j p) d -> p j d", p=128))
                        vts[0] = vt

            # ---------------- Phase B+C: attention (qc-outer) with the
            # out-projection of the previous q-chunk interleaved into the
            # PE stream as filler for the exp-bound stretches ----------------
            with tc.tile_pool(name="pBw", bufs=1) as pbw, \
                 tc.tile_pool(name="pBv23", bufs=1) as pbv23, \
                 tc.tile_pool(name="pBy", bufs=2) as pby, \
                 tc.tile_pool(name="pCo", bufs=4) as pco:
                for hp in (1, 2, 3):
                    vn, c0 = (vnat0, 256) if hp == 1 else (vnat1, (hp - 2) * 256)
                    vt = pbv23.tile([128, KT, 256], BF, name=f"vt{hp}",
                                    tag=f"vt{hp}")
                    nc.sync.dma_start(
                        out=vt,
                        in_=vn[:, c0:c0 + 256].rearrange(
                            "(j p) d -> p j d", p=128))
                    vts[hp] = vt
                # phase-C weights: loaded early in B (after A frees)
                wos = []
                for dd in range(NH):
                    wo = pbw.tile([128, D], BF, name=f"wo{dd}", tag=f"wo{dd}")
                    nc.sync.dma_start(
                        out=wo, in_=woT[dd * 128:(dd + 1) * 128, :])
                    wos.append(wo)

                with tc.tile_pool(name="psS", bufs=2, space="PSUM") as pss, \
                     tc.tile_pool(name="psY", bufs=2, space="PSUM") as psy, \
                     tc.tile_pool(name="psC", bufs=2, space="PSUM") as psc:
                    # deferred per-(h, qc) epilogue so the denominator chain
                    # never blocks the PE FIFO
                    pending = [None]

                    def finalize():
                        if pending[0] is None:
                            return
                        ypsum_f, acc_f, h_f, yq_f, qc_f = pending[0]
                        pending[0] = None
                        if qc_f == 0:
                            # close the psum accumulation group on columns
                            # the truncated diagonal tiles never re-touched
                            nc.tensor.matmul(
                                ypsum_f[:, 0:384], ones128, zeros[:, 0:384],
                                start=False, stop=True)
                        dps = pss.tile([128, 2, 512], F32, name="dps", tag="sc")
                        nc.tensor.matmul(dps[:, 0, :], ones128, acc_f,
                                         start=True, stop=True)
                        rbs = pbr.tile([128, 512], BF, name="rbs", tag="rbs")
                        with nc.allow_low_precision("softmax recip bf16"):
                            nc.vector.reciprocal(out=rbs, in_=dps[:, 0, :])
                        nc.vector.tensor_mul(yq_f[:, h_f, :], ypsum_f, rbs)

                    def c_gen(yq, qcc):
                        # out-projection of q-chunk qcc, one PE matmul per
                        # yield step
                        for e in range(DT):
                            op = psc.tile([128, 512], F32, name="op", tag="op")
                            for dd in range(NH):
                                nc.tensor.matmul(
                                    op,
                                    wos[dd][:, e * 128:(e + 1) * 128],
                                    yq[:, dd, :],
                                    start=(dd == 0), stop=(dd == NH - 1))
                                yield
                            ot = pco.tile([128, 512], BF, name="ot", tag="ot")
                            nc.scalar.copy(out=ot, in_=op)
                            nc.sync.dma_start(
                                out=outT[e * 128:(e + 1) * 128,
                                         qcc * 512:(qcc + 1) * 512],
                                in_=ot)
                            yield

                    cg = [None]

                    def c_step(k):
                        if cg[0] is None:
                            return
                        for _ in range(k):
                            if next(cg[0], "done") == "done":
                                cg[0] = None
                                return

                    for qc in range(QC if "B" in phases else 0):
                        yq = pby.tile([128, NH, 512], BF, name="yq", tag="yq")
                        for h in range(NH):
                            hp, h2 = h // 2, h % 2
                            vt = vts[hp]
                            # entries: diagonal k-tiles as singles (truncated
                            # widths), below-diagonal k-tiles paired
                            # two-per-exp to amortize ScalarE access latency
                            entries = [[(4 * qc + m, 128 * m)]
                                       for m in range(4)]
                            entries += [[(j, 0), (j + 1, 0)]
                                        for j in range(0, 4 * qc, 2)]
                            n = len(entries)
                            ypsum = psy.tile([128, 512], F32, name="ypsum",
                                             tag="yp")
                            acc = pac.tile([128, 512], BF, name="acc",
                                           tag="acc")
                            ats = {}

                            def emit(ei, h=h, qc=qc, entries=entries,
                                     acc=acc, ats=ats):
                                slots = entries[ei]
                                sc = pss.tile([128, 2, 512], F32,
                                              name="sc", tag="sc")
                                for s, (jg, c0) in enumerate(slots):
                                    nc.tensor.matmul(
                                        sc[:, s, c0:512],
                                        kT[:, h, jg * 128:(jg + 1) * 128],
                                        qT[:, h,
                                           qc * 512 + c0:(qc + 1) * 512],
                                        start=True, stop=True)
                                at = pba.tile([128, 2, 512], BF,
                                              name="at", tag="at")
                                if len(slots) == 2:
                                    nc.scalar.activation(
                                        out=at, in_=sc, func=EXP,
                                        scale=ALPHA)
                                else:
                                    jg, c0 = slots[0]
                                    nc.scalar.activation(
                                        out=at[:, 0, c0:512],
                                        in_=sc[:, 0, c0:512],
                                        func=EXP, scale=ALPHA)
                                    nc.vector.tensor_mul(
                                        at[:, 0, c0:c0 + 128],
                                        at[:, 0, c0:c0 + 128], mtri)
                                if ei == 0:
                                    nc.vector.tensor_copy(out=acc,
                                                          in_=at[:, 0, :])
                                else:
                                    for s, (jg, c0) in enumerate(slots):
                                        nc.vector.tensor_add(
                                            acc[:, c0:512],
                                            acc[:, c0:512],
                                            at[:, s, c0:512])
                                ats[ei] = (at, slots)

                            for ei in range(min(la, n)):
                                emit(ei)
                            finalize()
                            for ei in range(n):
                                if ei + la < n:
                                    emit(ei + la)
                                at, slots = ats.pop(ei)
                                for s, (jg, c0) in enumerate(slots):
                                    nc.tensor.matmul(
                                        ypsum[:, c0:512],
                                        vt[:, jg,
                                           h2 * 128:(h2 + 1) * 128],
                                        at[:, s, c0:512],
                                        start=(ei == 0 and s == 0),
                                        stop=(ei == n - 1 and
                                              s == len(slots) - 1))
                                c_step(3)
                            pending[0] = (ypsum, acc, h, yq, qc)
                        finalize()
                        c_step(200)  # drain any leftover out-proj steps
                        cg[0] = c_gen(yq, qc)
                    c_step(200)

            pbr.release()
            pac.release()
            pba.release()
            pbv01.release()
    nc.compile()
    return nc


_NC_CACHE = None


def _get_program():
    global _NC_CACHE
    if _NC_CACHE is None:
        _NC_CACHE = build_program()
    return _NC_CACHE


def _host_inputs(x, w_qkv, w_o):
    inv = 1.0 / (ROPE_BASE ** (np.arange(0, HD, 2, dtype=np.float64) / HD))
    ang = np.arange(L, dtype=np.float64)[:, None] * inv[None, :]
    chalf = np.tile(np.cos(ang), (1, 4)).astype(BF16)          # [L, 256]
    shalf = np.tile(np.sin(ang), (1, 4)).astype(BF16)
    p = np.arange(128)[:, None]
    f = np.arange(128)[None, :]
    mtri = (p <= f).astype(BF16)                               # [128, 128]
    ident = np.eye(128).astype(BF16)
    # per-head [even dims | odd dims] permutation for q/k rows
    perm = np.concatenate([np.arange(0, 128, 2), np.arange(1, 128, 2)])

    in_maps = []
    for c in range(8):
        b, g = c % 4, c // 4
        qr = w_qkv[g * DL:(g + 1) * DL]
        kr = w_qkv[D + g * DL:D + (g + 1) * DL]
        vr = w_qkv[2 * D + g * DL:2 * D + (g + 1) * DL]
        qp = qr.reshape(NH, HD, D)[:, perm, :].reshape(DL, D)
        kp = kr.reshape(NH, HD, D)[:, perm, :].reshape(DL, D)
        wqkvT = np.ascontiguousarray(
            np.concatenate([qp[:512], kp[:512], vr[:512],
                            qp[512:], kp[512:], vr[512:]], axis=0).T
        ).astype(BF16)
        in_maps.append({
            "xT": np.ascontiguousarray(x[b].T).astype(BF16),
            "wqkvT": wqkvT,
            "woT": np.ascontiguousarray(
                w_o[:, g * DL:(g + 1) * DL].T).astype(BF16),
            "chalf": chalf,
            "shalf": shalf,
            "mtri": mtri,
            "ident": ident,
        })
    return in_maps


def kernel(x, w_qkv, w_o, _trace=False):
    x = np.asarray(x, dtype=np.float32)
    w_qkv = np.asarray(w_qkv, dtype=np.float32)
    w_o = np.asarray(w_o, dtype=np.float32)
    nc = _get_program()
    in_maps = _host_inputs(x, w_qkv, w_o)
    res = run_bass_kernel_spmd(nc, in_maps, core_ids=list(range(8)),
                               trace=_trace)
    kernel.last_result = res
    parts = [np.asarray(r["outT"], dtype=np.float32) for r in res.results]
    out = np.empty((B, L, D), dtype=np.float32)
    for b in range(B):
        out[b] = (parts[b] + parts[b + 4]).T
    return out


# revision 15
# speedup vs baseline: 1.1146x; 1.0113x over previous
"""MHA (RoPE + causal softmax attention + out-proj) on 8 NeuronCores.

Sharding: DP4 x TP2. Core c: batch b = c % 4, head-group g = c // 4
(8 heads per core). Each core computes a transposed partial output
outT = (y_local @ w_o_slice^T)^T in [D, L] (bf16); host sums the two
head-group partials per batch and transposes back.

All matmuls bf16 x bf16 -> fp32 PSUM. Layout strategy (v2):
  Phase A: qkv natural layout [L, comps] via out = xT_tile.T @ w_chunk,
           d-outer accumulation over 6-tile L-groups so the PE starts as
           soon as the first (x, w) d-pair lands. q/k head-dims are
           pre-permuted host-side to [even|odd] halves so RoPE uses
           packed (unstrided) APs: one ScalarE copy PSUM->bf16, then 6
           DVE ops. Rotated q/k tiles are transposed on the PE
           (128x128 transpose-matmuls -> bf16 PSUM -> one DVE copy)
           into SBUF-resident qT/kT [hd, L] per head - no DRAM round
           trip, no DMA-xbar transposes. v is staged to DRAM (natural
           layout) via gpsimd-issued DMAs.
  Phase B: per head, scores computed transposed (k on partitions) from
           SBUF qT/kT; exp on ScalarE with the 1/sqrt(HD) scale fused.
           Causal: fully-masked k-tiles skipped, diagonal k-tiles
           truncated to the valid q-range (matmul cols 512/384/256/128),
           single [128,128] triangle mask. Softmax denominator via DVE
           bf16 accumulation of the exp tiles + ONE all-ones matmul per
           (head, q-chunk) instead of a ones-matmul per k-tile.
  Phase C: out-proj outT[e, q] = sum_d w_oT[d, e] * yT[d, q], bf16 out.
"""

import numpy as np
import ml_dtypes

import concourse.bass as bass
import concourse.tile as tile
import concourse.mybir as mybir
from concourse import bacc
from concourse.bass_utils import run_bass_kernel_spmd

BF16 = ml_dtypes.bfloat16
F32 = mybir.dt.float32
BF = mybir.dt.bfloat16

B, L, D, H, HD = 4, 2048, 2048, 16, 128
NH = 8                      # heads per core
DL = NH * HD                # 1024 local head dims
ROPE_BASE = 10000.0
ALPHA = float(HD) ** -0.5

LT = L // 128               # 16 L-tiles
DT = D // 128               # 16 D(contract)-tiles
NCH = 6                     # qkv chunks of 512 comps: q03,k03,v03,q47,k47,v47
QC = L // 512               # 4 q-chunks of 512
KT = L // 128               # 16 k-tiles
GROUPS = [(0, 6), (6, 12), (12, 16)]

EXP = mybir.ActivationFunctionType.Exp


CHUNKS = [("q", 0), ("k", 0), ("v", 0), ("v", 1), ("q", 1), ("k", 1)]


def _chunk_kind(c):
    # chunk order: q(heads0-3), k(0-3), v(0-3), v(4-7), q(4-7), k(4-7)
    return CHUNKS[c]


def build_program(phases="ABC", la=1):
    nc = bacc.Bacc("TRN2", target_bir_lowering=False, debug=False, num_devices=8)

    xT = nc.dram_tensor("xT", [D, L], BF, kind="ExternalInput").ap()
    wqkvT = nc.dram_tensor("wqkvT", [D, 3 * DL], BF, kind="ExternalInput").ap()
    woT = nc.dram_tensor("woT", [DL, D], BF, kind="ExternalInput").ap()
    chalf = nc.dram_tensor("chalf", [L, 64], BF, kind="ExternalInput").ap()
    shalf = nc.dram_tensor("shalf", [L, 64], BF, kind="ExternalInput").ap()
    mtri_d = nc.dram_tensor("mtri", [128, 128], BF, kind="ExternalInput").ap()
    ident_d = nc.dram_tensor("ident", [128, 128], BF, kind="ExternalInput").ap()
    outT = nc.dram_tensor("outT", [D, L], BF, kind="ExternalOutput").ap()

    # DRAM staging for v (natural layout), split per head-group so the
    # vt reloads for heads 0-3 have no dependency on the group-1 writes
    vnat0 = nc.dram_tensor("vnat0", [L, 512], BF, kind="Internal").ap()
    vnat1 = nc.dram_tensor("vnat1", [L, 512], BF, kind="Internal").ap()

    with tile.TileContext(nc) as tc:
        with tc.tile_pool(name="outer", bufs=1) as outer:
            # persistent: transposed rotated q/k per head [hd, L]
            qT = outer.tile([128, NH, L], BF, name="qT", tag="qT")
            kT = outer.tile([128, NH, L], BF, name="kT", tag="kT")
            c_sb = outer.tile([128, LT, 1, 64], BF, name="c_sb", tag="c_sb")
            s_sb = outer.tile([128, LT, 1, 64], BF, name="s_sb", tag="s_sb")
            ones128 = outer.tile([128, 128], BF, name="ones128", tag="oc")
            zeros = outer.tile([128, 512], BF, name="zeros", tag="zc")
            mtri = outer.tile([128, 128], BF, name="mtri", tag="mtri")
            idt = outer.tile([128, 128], BF, name="idt", tag="idt")
            nc.vector.memset(ones128, 1.0)
            nc.vector.memset(zeros, 0.0)
            if "B" not in phases:
                nc.vector.memset(qT, 0.0)
                nc.vector.memset(kT, 0.0)

            # small phase-B pools opened FIRST so their SBUF addresses do
            # not overlap the phase-A zone (no release dependency on the
            # phase-A drain)
            pbv = tc.alloc_tile_pool(name="pBv", bufs=1)
            pba = tc.alloc_tile_pool(name="pBa", bufs=3)
            pac = tc.alloc_tile_pool(name="pBc", bufs=2)
            pbr = tc.alloc_tile_pool(name="pBr", bufs=1)
            vts = [None] * 4

            # ---------------- Phase A: QKV + RoPE + transpose ----------------
            with tc.tile_pool(name="pA", bufs=1) as pa, \
                 tc.tile_pool(name="pAw", bufs=2) as paw, \
                 tc.tile_pool(name="pAt", bufs=1) as pat, \
                 tc.tile_pool(name="pAo", bufs=4) as pao, \
                 tc.tile_pool(name="pAr", bufs=2) as pro, \
                 tc.tile_pool(name="psA", bufs=1, space="PSUM") as psa, \
                 tc.tile_pool(name="psT", bufs=2, space="PSUM") as pst:
                # PE warm-up: a few dummy matmuls so the p-state ramp runs
                # during the initial DMA loads instead of on real work
                warm = psa.tile([128, 512], F32, name="pn", tag="pn0")
                for _ in range(8):
                    nc.tensor.matmul(warm, ones128, zeros, start=True,
                                     stop=True)
                # resident xT tiles [128, L] per D-tile; loaded in column
                # halves interleaved with chunk-0 weight d-tiles so the
                # d-outer accumulation can start almost immediately.
                xts = [pa.tile([128, L], BF, name=f"xt{d}", tag=f"xt{d}")
                       for d in range(DT)]

                def load_wch_half(c, half):
                    wh = paw.tile([128, 8, 512], BF, name="wch", tag="wch")
                    for dd in range(8):
                        d = half * 8 + dd
                        nc.sync.dma_start(
                            out=wh[:, dd, :],
                            in_=wqkvT[d * 128:(d + 1) * 128,
                                      c * 512:(c + 1) * 512])
                    return wh

                wh0a = paw.tile([128, 8, 512], BF, name="wch", tag="wch")
                wh0b = paw.tile([128, 8, 512], BF, name="wch", tag="wch")
                for d in range(DT):
                    nc.sync.dma_start(out=xts[d][:, 0:1024],
                                      in_=xT[d * 128:(d + 1) * 128, 0:1024])
                    wh = wh0a if d < 8 else wh0b
                    nc.sync.dma_start(
                        out=wh[:, d % 8, :],
                        in_=wqkvT[d * 128:(d + 1) * 128, 0:512])
                nc.sync.dma_start(
                    out=c_sb,
                    in_=chalf.rearrange("(i p) g -> p i g", p=128))
                nc.sync.dma_start(
                    out=s_sb,
                    in_=shalf.rearrange("(i p) g -> p i g", p=128))
                nc.sync.dma_start(out=idt, in_=ident_d)
                nc.sync.dma_start(out=mtri, in_=mtri_d)
                for d in range(DT):
                    nc.sync.dma_start(out=xts[d][:, 1024:2048],
                                      in_=xT[d * 128:(d + 1) * 128, 1024:2048])

                for c in range(NCH if "A" in phases else 0):
                    kind, grp = _chunk_kind(c)
                    if c == 0:
                        wha, whb = wh0a, wh0b
                    else:
                        wha = load_wch_half(c, 0)
                        whb = load_wch_half(c, 1)
                    for (lo, hi) in GROUPS:
                        pns = [psa.tile([128, 512], F32, name="pn", tag=f"pn{i - lo}")
                               for i in range(lo, hi)]
                        for d in range(DT):
                            for i in range(lo, hi):
                                wh = wha if d < 8 else whb
                                nc.tensor.matmul(
                                    pns[i - lo],
                                    xts[d][:, i * 128:(i + 1) * 128],
                                    wh[:, d % 8, :],
                                    start=(d == 0), stop=(d == DT - 1))
                        for i in range(lo, hi):
                            pn = pns[i - lo]
                            if kind == "v":
                                vo = pao.tile([128, 512], BF, name="vo", tag="vo")
                                nc.scalar.copy(out=vo, in_=pn)
                                vn = vnat0 if grp == 0 else vnat1
                                nc.gpsimd.dma_start(
                                    out=vn[i * 128:(i + 1) * 128, :],
                                    in_=vo)
                            else:
                                xb = pao.tile([128, 4, 128], BF, name="xb", tag="vo")
                                nc.scalar.copy(out=xb, in_=pn)
                                x1 = xb[:, :, 0:64]
                                x2 = xb[:, :, 64:128]
                                ct = c_sb[:, i].broadcast_to([128, 4, 64])
                                st = s_sb[:, i].broadcast_to([128, 4, 64])
                                t1 = pat.tile([128, 4, 64], BF, name="t1", tag="t1")
                                nc.vector.tensor_mul(t1, x1, ct)
                                t2 = pat.tile([128, 4, 64], BF, name="t2", tag="t2")
                                nc.vector.tensor_mul(t2, x2, st)
                                t3 = pat.tile([128, 4, 64], BF, name="t3", tag="t3")
                                nc.vector.tensor_mul(t3, x2, ct)
                                t4 = pat.tile([128, 4, 64], BF, name="t4", tag="t4")
                                nc.vector.tensor_mul(t4, x1, st)
                                ro = pro.tile([128, 4, 128], BF, name="ro", tag="ro")
                                nc.vector.tensor_sub(ro[:, :, 0:64], t1, t2)
                                nc.vector.tensor_add(ro[:, :, 64:128], t3, t4)
                                pt = pst.tile([128, 4, 128], BF, name="pt", tag="pt")
                                for hh in range(4):
                                    nc.tensor.transpose(
                                        pt[:, hh, :], ro[:, hh, :], idt)
                                dstT = qT if kind == "q" else kT
                                nc.vector.tensor_copy(
                                    out=dstT[:, grp * 4:(grp + 1) * 4,
                                             i * 128:(i + 1) * 128],
                                    in_=pt)
                    if kind == "v" and "B" in phases:
                        # this head-group's v fully staged: prefetch its vt
                        vn = vnat0 if grp == 0 else vnat1
                        for hp in (2 * grp, 2 * grp + 1):
                            vt = pbv.tile([128, KT, 256], BF,
                                          name=f"vt{hp}", tag=f"vt{hp}")
                            nc.sync.dma_start(
                                out=vt,
                                in_=vn[:, (hp % 2) * 256:(hp % 2) * 256 + 256]
                                    .rearrange("(j p) d -> p j d", p=128))
                            vts[hp] = vt

            # ---------------- Phase B+C: attention (qc-outer) with the
            # out-projection of the previous q-chunk interleaved into the
            # PE stream as filler for the exp-bound stretches ----------------
            with tc.tile_pool(name="pBw", bufs=1) as pbw, \
                 tc.tile_pool(name="pBy", bufs=2) as pby, \
                 tc.tile_pool(name="pCo", bufs=4) as pco:
                # phase-C weights: loaded early in B (after A frees)
                wos = []
                for dd in range(NH):
                    wo = pbw.tile([128, D], BF, name=f"wo{dd}", tag=f"wo{dd}")
                    nc.sync.dma_start(
                        out=wo, in_=woT[dd * 128:(dd + 1) * 128, :])
                    wos.append(wo)

                with tc.tile_pool(name="psS", bufs=2, space="PSUM") as pss, \
                     tc.tile_pool(name="psY", bufs=2, space="PSUM") as psy, \
                     tc.tile_pool(name="psC", bufs=2, space="PSUM") as psc:
                    # deferred per-(h, qc) epilogue so the denominator chain
                    # never blocks the PE FIFO
                    pending = [None]

                    def finalize():
                        if pending[0] is None:
                            return
                        ypsum_f, acc_f, h_f, yq_f, qc_f = pending[0]
                        pending[0] = None
                        if qc_f == 0:
                            # close the psum accumulation group on columns
                            # the truncated diagonal tiles never re-touched
                            nc.tensor.matmul(
                                ypsum_f[:, 0:384], ones128, zeros[:, 0:384],
                                start=False, stop=True)
                        dps = pss.tile([128, 2, 512], F32, name="dps", tag="sc")
                        nc.tensor.matmul(dps[:, 0, :], ones128, acc_f,
                                         start=True, stop=True)
                        rbs = pbr.tile([128, 512], BF, name="rbs", tag="rbs")
                        with nc.allow_low_precision("softmax recip bf16"):
                            nc.vector.reciprocal(out=rbs, in_=dps[:, 0, :])
                        nc.vector.tensor_mul(yq_f[:, h_f, :], ypsum_f, rbs)

                    def c_gen(yq, qcc):
                        # out-projection of q-chunk qcc, one PE matmul per
                        # yield step
                        for e in range(DT):
                            op = psc.tile([128, 512], F32, name="op", tag="op")
                            for dd in range(NH):
                                nc.tensor.matmul(
                                    op,
                                    wos[dd][:, e * 128:(e + 1) * 128],
                                    yq[:, dd, :],
                                    start=(dd == 0), stop=(dd == NH - 1))
                                yield
                            ot = pco.tile([128, 512], BF, name="ot", tag="ot")
                            nc.vector.tensor_copy(out=ot, in_=op)
                            nc.sync.dma_start(
                                out=outT[e * 128:(e + 1) * 128,
                                         qcc * 512:(qcc + 1) * 512],
                                in_=ot)
                            yield

                    cg = [None]

                    def c_step(k):
                        if cg[0] is None:
                            return
                        for _ in range(k):
                            if next(cg[0], "done") == "done":
                                cg[0] = None
                                return

                    for qc in range(QC if "B" in phases else 0):
                        yq = pby.tile([128, NH, 512], BF, name="yq", tag="yq")
                        for h in range(NH):
                            hp, h2 = h // 2, h % 2
                            vt = vts[hp]
                            # entries: diagonal k-tiles as singles (truncated
                            # widths), below-diagonal k-tiles paired
                            # two-per-exp to amortize ScalarE access latency
                            entries = [[(4 * qc + m, 128 * m)]
                                       for m in range(4)]
                            entries += [[(j, 0), (j + 1, 0)]
                                        for j in range(0, 4 * qc, 2)]
                            n = len(entries)
                            ypsum = psy.tile([128, 512], F32, name="ypsum",
                                             tag="yp")
                            acc = pac.tile([128, 512], BF, name="acc",
                                           tag="acc")
                            ats = {}

                            def emit(ei, h=h, qc=qc, entries=entries,
                                     acc=acc, ats=ats):
                                slots = entries[ei]
                                sc = pss.tile([128, 2, 512], F32,
                                              name="sc", tag="sc")
                                for s, (jg, c0) in enumerate(slots):
                                    nc.tensor.matmul(
                                        sc[:, s, c0:512],
                                        kT[:, h, jg * 128:(jg + 1) * 128],
                                        qT[:, h,
                                           qc * 512 + c0:(qc + 1) * 512],
                                        start=True, stop=True)
                                at = pba.tile([128, 2, 512], BF,
                                              name="at", tag="at")
                                if len(slots) == 2:
                                    nc.scalar.activation(
                                        out=at, in_=sc, func=EXP,
                                        scale=ALPHA)
                                else:
                                    jg, c0 = slots[0]
                                    nc.scalar.activation(
                                        out=at[:, 0, c0:512],
                                        in_=sc[:, 0, c0:512],
                                        func=EXP, scale=ALPHA)
                                    nc.vector.tensor_mul(
                                        at[:, 0, c0:c0 + 128],
                                        at[:, 0, c0:c0 + 128], mtri)
                                if ei == 0:
                                    nc.vector.tensor_copy(out=acc,
                                                          in_=at[:, 0, :])
                                else:
                                    for s, (jg, c0) in enumerate(slots):
                                        nc.vector.tensor_add(
                                            acc[:, c0:512],
                                            acc[:, c0:512],
                                            at[:, s, c0:512])
                                ats[ei] = (at, slots)

                            for ei in range(min(la, n)):
                                emit(ei)
                            finalize()
                            for ei in range(n):
                                if ei + la < n:
                                    emit(ei + la)
                                at, slots = ats.pop(ei)
                                for s, (jg, c0) in enumerate(slots):
                                    nc.tensor.matmul(
                                        ypsum[:, c0:512],
                                        vt[:, jg,
                                           h2 * 128:(h2 + 1) * 128],
                                        at[:, s, c0:512],
                                        start=(ei == 0 and s == 0),
                                        stop=(ei == n - 1 and
                                              s == len(slots) - 1))
                                c_step(3)
                            pending[0] = (ypsum, acc, h, yq, qc)
                        finalize()
                        c_step(200)  # drain any leftover out-proj steps
                        cg[0] = c_gen(yq, qc)
                    c_step(200)

            pbr.release()
            pac.release()
            pba.release()
            pbv.release()
    nc.compile()
    return nc


_NC_CACHE = None


def _get_program():
    global _NC_CACHE
    if _NC_CACHE is None:
        _NC_CACHE = build_program()
    return _NC_CACHE


def _host_inputs(x, w_qkv, w_o):
    inv = 1.0 / (ROPE_BASE ** (np.arange(0, HD, 2, dtype=np.float64) / HD))
    ang = np.arange(L, dtype=np.float64)[:, None] * inv[None, :]
    chalf = np.cos(ang).astype(BF16)                           # [L, 64]
    shalf = np.sin(ang).astype(BF16)
    p = np.arange(128)[:, None]
    f = np.arange(128)[None, :]
    mtri = (p <= f).astype(BF16)                               # [128, 128]
    ident = np.eye(128).astype(BF16)
    # per-head [even dims | odd dims] permutation for q/k rows
    perm = np.concatenate([np.arange(0, 128, 2), np.arange(1, 128, 2)])

    in_maps = []
    for c in range(8):
        b, g = c % 4, c // 4
        qr = w_qkv[g * DL:(g + 1) * DL]
        kr = w_qkv[D + g * DL:D + (g + 1) * DL]
        vr = w_qkv[2 * D + g * DL:2 * D + (g + 1) * DL]
        qp = qr.reshape(NH, HD, D)[:, perm, :].reshape(DL, D)
        kp = kr.reshape(NH, HD, D)[:, perm, :].reshape(DL, D)
        wqkvT = np.ascontiguousarray(
            np.concatenate([qp[:512], kp[:512], vr[:512],
                            vr[512:], qp[512:], kp[512:]], axis=0).T
        ).astype(BF16)
        in_maps.append({
            "xT": np.ascontiguousarray(x[b].T).astype(BF16),
            "wqkvT": wqkvT,
            "woT": np.ascontiguousarray(
                w_o[:, g * DL:(g + 1) * DL].T).astype(BF16),
            "chalf": chalf,
            "shalf": shalf,
            "mtri": mtri,
            "ident": ident,
        })
    return in_maps


def kernel(x, w_qkv, w_o, _trace=False):
    x = np.asarray(x, dtype=np.float32)
    w_qkv = np.asarray(w_qkv, dtype=np.float32)
    w_o = np.asarray(w_o, dtype=np.float32)
    nc = _get_program()
    in_maps = _host_inputs(x, w_qkv, w_o)
    res = run_bass_kernel_spmd(nc, in_maps, core_ids=list(range(8)),
                               trace=_trace)
    kernel.last_result = res
    parts = [np.asarray(r["outT"], dtype=np.float32) for r in res.results]
    out = np.empty((B, L, D), dtype=np.float32)
    for b in range(B):
        out[b] = (parts[b] + parts[b + 4]).T
    return out


# revision 17
# speedup vs baseline: 1.1264x; 1.0106x over previous
"""MHA (RoPE + causal softmax attention + out-proj) on 8 NeuronCores.

Sharding: DP4 x TP2. Core c: batch b = c % 4, head-group g = c // 4
(8 heads per core). Each core computes a transposed partial output
outT = (y_local @ w_o_slice^T)^T in [D, L] (bf16); host sums the two
head-group partials per batch and transposes back.

All matmuls bf16 x bf16 -> fp32 PSUM. Layout strategy (v2):
  Phase A: qkv natural layout [L, comps] via out = xT_tile.T @ w_chunk,
           d-outer accumulation over 6-tile L-groups so the PE starts as
           soon as the first (x, w) d-pair lands. q/k head-dims are
           pre-permuted host-side to [even|odd] halves so RoPE uses
           packed (unstrided) APs: one ScalarE copy PSUM->bf16, then 6
           DVE ops. Rotated q/k tiles are transposed on the PE
           (128x128 transpose-matmuls -> bf16 PSUM -> one DVE copy)
           into SBUF-resident qT/kT [hd, L] per head - no DRAM round
           trip, no DMA-xbar transposes. v is staged to DRAM (natural
           layout) via gpsimd-issued DMAs.
  Phase B: per head, scores computed transposed (k on partitions) from
           SBUF qT/kT; exp on ScalarE with the 1/sqrt(HD) scale fused.
           Causal: fully-masked k-tiles skipped, diagonal k-tiles
           truncated to the valid q-range (matmul cols 512/384/256/128),
           single [128,128] triangle mask. Softmax denominator via DVE
           bf16 accumulation of the exp tiles + ONE all-ones matmul per
           (head, q-chunk) instead of a ones-matmul per k-tile.
  Phase C: out-proj outT[e, q] = sum_d w_oT[d, e] * yT[d, q], bf16 out.
"""

import numpy as np
import ml_dtypes

import concourse.bass as bass
import concourse.tile as tile
import concourse.mybir as mybir
from concourse import bacc
from concourse.bass_utils import run_bass_kernel_spmd

BF16 = ml_dtypes.bfloat16
F32 = mybir.dt.float32
BF = mybir.dt.bfloat16

B, L, D, H, HD = 4, 2048, 2048, 16, 128
NH = 8                      # heads per core
DL = NH * HD                # 1024 local head dims
ROPE_BASE = 10000.0
ALPHA = float(HD) ** -0.5

LT = L // 128               # 16 L-tiles
DT = D // 128               # 16 D(contract)-tiles
NCH = 6                     # qkv chunks of 512 comps: q03,k03,v03,q47,k47,v47
QC = L // 512               # 4 q-chunks of 512
KT = L // 128               # 16 k-tiles
GROUPS = [(0, 6), (6, 12), (12, 16)]

EXP = mybir.ActivationFunctionType.Exp


CHUNKS = [("q", 0), ("k", 0), ("v", 0), ("v", 1), ("q", 1), ("k", 1)]


def _chunk_kind(c):
    # chunk order: q(heads0-3), k(0-3), v(0-3), v(4-7), q(4-7), k(4-7)
    return CHUNKS[c]


def build_program(phases="ABC", la=1):
    nc = bacc.Bacc("TRN2", target_bir_lowering=False, debug=False, num_devices=8)

    xT = nc.dram_tensor("xT", [D, L], BF, kind="ExternalInput").ap()
    wqkvT = nc.dram_tensor("wqkvT", [D, 3 * DL], BF, kind="ExternalInput").ap()
    woT = nc.dram_tensor("woT", [DL, D], BF, kind="ExternalInput").ap()
    chalf = nc.dram_tensor("chalf", [L, 64], BF, kind="ExternalInput").ap()
    shalf = nc.dram_tensor("shalf", [L, 64], BF, kind="ExternalInput").ap()
    mtri_d = nc.dram_tensor("mtri", [128, 128], BF, kind="ExternalInput").ap()
    ident_d = nc.dram_tensor("ident", [128, 128], BF, kind="ExternalInput").ap()
    outT = nc.dram_tensor("outT", [D, L], BF, kind="ExternalOutput").ap()

    # DRAM staging for v (natural layout), split per head-group so the
    # vt reloads for heads 0-3 have no dependency on the group-1 writes
    vnat0 = nc.dram_tensor("vnat0", [L, 512], BF, kind="Internal").ap()
    vnat1 = nc.dram_tensor("vnat1", [L, 512], BF, kind="Internal").ap()

    with tile.TileContext(nc) as tc:
        with tc.tile_pool(name="outer", bufs=1) as outer:
            # persistent: transposed rotated q/k per head [hd, L]
            qT = outer.tile([128, NH, L], BF, name="qT", tag="qT")
            kT = outer.tile([128, NH, L], BF, name="kT", tag="kT")
            c_sb = outer.tile([128, LT, 1, 64], BF, name="c_sb", tag="c_sb")
            s_sb = outer.tile([128, LT, 1, 64], BF, name="s_sb", tag="s_sb")
            ones128 = outer.tile([128, 128], BF, name="ones128", tag="oc")
            zeros = outer.tile([128, 512], BF, name="zeros", tag="zc")
            mtri = outer.tile([128, 128], BF, name="mtri", tag="mtri")
            idt = outer.tile([128, 128], BF, name="idt", tag="idt")
            nc.vector.memset(ones128, 1.0)
            nc.vector.memset(zeros, 0.0)
            if "B" not in phases:
                nc.vector.memset(qT, 0.0)
                nc.vector.memset(kT, 0.0)

            # small phase-B pools opened FIRST so their SBUF addresses do
            # not overlap the phase-A zone (no release dependency on the
            # phase-A drain)
            pbv = tc.alloc_tile_pool(name="pBv", bufs=1)
            pba = tc.alloc_tile_pool(name="pBa", bufs=3)
            pac = tc.alloc_tile_pool(name="pBc", bufs=2)
            pbr = tc.alloc_tile_pool(name="pBr", bufs=1)
            vts = [None] * 4

            # ---------------- Phase A: QKV + RoPE + transpose ----------------
            with tc.tile_pool(name="pA", bufs=1) as pa, \
                 tc.tile_pool(name="pAw", bufs=2) as paw, \
                 tc.tile_pool(name="pAt", bufs=1) as pat, \
                 tc.tile_pool(name="pAo", bufs=4) as pao, \
                 tc.tile_pool(name="pAr", bufs=2) as pro, \
                 tc.tile_pool(name="psA", bufs=1, space="PSUM") as psa, \
                 tc.tile_pool(name="psT", bufs=2, space="PSUM") as pst:
                # PE warm-up: a few dummy matmuls so the p-state ramp runs
                # during the initial DMA loads instead of on real work
                warm = psa.tile([128, 512], F32, name="pn", tag="pn0")
                for _ in range(8):
                    nc.tensor.matmul(warm, ones128, zeros, start=True,
                                     stop=True)
                # resident xT tiles [128, L] per D-tile; loaded in column
                # halves interleaved with chunk-0 weight d-tiles so the
                # d-outer accumulation can start almost immediately.
                xts = [pa.tile([128, L], BF, name=f"xt{d}", tag=f"xt{d}")
                       for d in range(DT)]

                def load_wch_half(c, half):
                    wh = paw.tile([128, 8, 512], BF, name="wch", tag="wch")
                    for dd in range(8):
                        d = half * 8 + dd
                        nc.sync.dma_start(
                            out=wh[:, dd, :],
                            in_=wqkvT[d * 128:(d + 1) * 128,
                                      c * 512:(c + 1) * 512])
                    return wh

                wh0a = paw.tile([128, 8, 512], BF, name="wch", tag="wch")
                wh0b = paw.tile([128, 8, 512], BF, name="wch", tag="wch")
                for d in range(DT):
                    nc.sync.dma_start(out=xts[d][:, 0:1024],
                                      in_=xT[d * 128:(d + 1) * 128, 0:1024])
                    wh = wh0a if d < 8 else wh0b
                    nc.sync.dma_start(
                        out=wh[:, d % 8, :],
                        in_=wqkvT[d * 128:(d + 1) * 128, 0:512])
                nc.sync.dma_start(
                    out=c_sb,
                    in_=chalf.rearrange("(i p) g -> p i g", p=128))
                nc.sync.dma_start(
                    out=s_sb,
                    in_=shalf.rearrange("(i p) g -> p i g", p=128))
                nc.sync.dma_start(out=idt, in_=ident_d)
                nc.sync.dma_start(out=mtri, in_=mtri_d)
                for d in range(DT):
                    nc.sync.dma_start(out=xts[d][:, 1024:2048],
                                      in_=xT[d * 128:(d + 1) * 128, 1024:2048])

                for c in range(NCH if "A" in phases else 0):
                    kind, grp = _chunk_kind(c)
                    if c == 0:
                        wha, whb = wh0a, wh0b
                    else:
                        wha = load_wch_half(c, 0)
                        whb = load_wch_half(c, 1)
                    for (lo, hi) in GROUPS:
                        pns = [psa.tile([128, 512], F32, name="pn", tag=f"pn{i - lo}")
                               for i in range(lo, hi)]
                        for d in range(DT):
                            for i in range(lo, hi):
                                wh = wha if d < 8 else whb
                                nc.tensor.matmul(
                                    pns[i - lo],
                                    xts[d][:, i * 128:(i + 1) * 128],
                                    wh[:, d % 8, :],
                                    start=(d == 0), stop=(d == DT - 1))
                        for i in range(lo, hi):
                            pn = pns[i - lo]
                            if kind == "v":
                                vo = pao.tile([128, 512], BF, name="vo", tag="vo")
                                nc.scalar.copy(out=vo, in_=pn)
                                vn = vnat0 if grp == 0 else vnat1
                                nc.gpsimd.dma_start(
                                    out=vn[i * 128:(i + 1) * 128, :],
                                    in_=vo)
                            else:
                                xb = pao.tile([128, 4, 128], BF, name="xb", tag="vo")
                                nc.scalar.copy(out=xb, in_=pn)
                                x1 = xb[:, :, 0:64]
                                x2 = xb[:, :, 64:128]
                                ct = c_sb[:, i].broadcast_to([128, 4, 64])
                                st = s_sb[:, i].broadcast_to([128, 4, 64])
                                t1 = pat.tile([128, 4, 64], BF, name="t1", tag="t1")
                                nc.vector.tensor_mul(t1, x1, ct)
                                t2 = pat.tile([128, 4, 64], BF, name="t2", tag="t2")
                                nc.vector.tensor_mul(t2, x2, st)
                                t3 = pat.tile([128, 4, 64], BF, name="t3", tag="t3")
                                nc.vector.tensor_mul(t3, x2, ct)
                                t4 = pat.tile([128, 4, 64], BF, name="t4", tag="t4")
                                nc.vector.tensor_mul(t4, x1, st)
                                ro = pro.tile([128, 4, 128], BF, name="ro", tag="ro")
                                nc.vector.tensor_sub(ro[:, :, 0:64], t1, t2)
                                nc.vector.tensor_add(ro[:, :, 64:128], t3, t4)
                                pt = pst.tile([128, 4, 128], BF, name="pt", tag="pt")
                                for hh in range(4):
                                    nc.tensor.transpose(
                                        pt[:, hh, :], ro[:, hh, :], idt)
                                dstT = qT if kind == "q" else kT
                                nc.vector.tensor_copy(
                                    out=dstT[:, grp * 4:(grp + 1) * 4,
                                             i * 128:(i + 1) * 128],
                                    in_=pt)
                    if kind == "v" and "B" in phases:
                        # this head-group's v fully staged: prefetch its vt
                        vn = vnat0 if grp == 0 else vnat1
                        for hp in (2 * grp, 2 * grp + 1):
                            vt = pbv.tile([128, KT, 256], BF,
                                          name=f"vt{hp}", tag=f"vt{hp}")
                            nc.sync.dma_start(
                                out=vt,
                                in_=vn[:, (hp % 2) * 256:(hp % 2) * 256 + 256]
                                    .rearrange("(j p) d -> p j d", p=128))
                            vts[hp] = vt

            # ---------------- Phase B+C: attention (qc-outer) with the
            # out-projection of the previous q-chunk interleaved into the
            # PE stream as filler for the exp-bound stretches ----------------
            with tc.tile_pool(name="pBw", bufs=1) as pbw, \
                 tc.tile_pool(name="pBy", bufs=2) as pby, \
                 tc.tile_pool(name="pCo", bufs=4) as pco:
                # phase-C weights: loaded early in B (after A frees)
                wos = []
                for dd in range(NH):
                    wo = pbw.tile([128, D], BF, name=f"wo{dd}", tag=f"wo{dd}")
                    nc.sync.dma_start(
                        out=wo, in_=woT[dd * 128:(dd + 1) * 128, :])
                    wos.append(wo)

                with tc.tile_pool(name="psS", bufs=2, space="PSUM") as pss, \
                     tc.tile_pool(name="psY", bufs=2, space="PSUM") as psy, \
                     tc.tile_pool(name="psC", bufs=2, space="PSUM") as psc:
                    # deferred per-(h, qc) epilogue so the denominator chain
                    # never blocks the PE FIFO
                    pending = [None]

                    def finalize():
                        if pending[0] is None:
                            return
                        ypsum_f, acc_f, h_f, yq_f, qc_f = pending[0]
                        pending[0] = None
                        if qc_f == 0:
                            # close the psum accumulation group on columns
                            # the truncated diagonal tiles never re-touched
                            nc.tensor.matmul(
                                ypsum_f[:, 0:384], ones128, zeros[:, 0:384],
                                start=False, stop=True)
                        dps = pss.tile([128, 2, 512], F32, name="dps", tag="sc")
                        nc.tensor.matmul(dps[:, 0, :], ones128, acc_f,
                                         start=True, stop=True)
                        rbs = pbr.tile([128, 512], BF, name="rbs", tag="rbs")
                        with nc.allow_low_precision("softmax recip bf16"):
                            nc.vector.reciprocal(out=rbs, in_=dps[:, 0, :])
                        nc.vector.tensor_mul(yq_f[:, h_f, :], ypsum_f, rbs)

                    def c_gen(yq, qcc):
                        # out-projection of q-chunk qcc, one PE matmul per
                        # yield step
                        for e in range(DT):
                            op = psc.tile([128, 512], F32, name="op", tag="op")
                            for dd in range(NH):
                                nc.tensor.matmul(
                                    op,
                                    wos[dd][:, e * 128:(e + 1) * 128],
                                    yq[:, dd, :],
                                    start=(dd == 0), stop=(dd == NH - 1))
                                yield
                            ot = pco.tile([128, 512], BF, name="ot", tag="ot")
                            nc.scalar.copy(out=ot, in_=op)
                            nc.sync.dma_start(
                                out=outT[e * 128:(e + 1) * 128,
                                         qcc * 512:(qcc + 1) * 512],
                                in_=ot)
                            yield

                    cg = [None]
                    credit = [0.0]

                    def c_step(k):
                        if cg[0] is None:
                            return
                        credit[0] += k
                        while credit[0] >= 1.0:
                            credit[0] -= 1.0
                            if next(cg[0], "done") == "done":
                                cg[0] = None
                                return

                    cpace = 0.0
                    for qc in range(QC if "B" in phases else 0):
                        yq = pby.tile([128, NH, 512], BF, name="yq", tag="yq")
                        for h in range(NH):
                            hp, h2 = h // 2, h % 2
                            vt = vts[hp]
                            # entries: diagonal k-tiles as singles (truncated
                            # widths), below-diagonal k-tiles paired
                            # two-per-exp to amortize ScalarE access latency
                            entries = [[(4 * qc + m, 128 * m)]
                                       for m in range(4)]
                            entries += [[(j, 0), (j + 1, 0)]
                                        for j in range(0, 4 * qc, 2)]
                            n = len(entries)
                            ypsum = psy.tile([128, 512], F32, name="ypsum",
                                             tag="yp")
                            acc = pac.tile([128, 512], BF, name="acc",
                                           tag="acc")
                            ats = {}

                            def emit(ei, h=h, qc=qc, entries=entries,
                                     acc=acc, ats=ats):
                                slots = entries[ei]
                                sc = pss.tile([128, 2, 512], F32,
                                              name="sc", tag="sc")
                                for s, (jg, c0) in enumerate(slots):
                                    nc.tensor.matmul(
                                        sc[:, s, c0:512],
                                        kT[:, h, jg * 128:(jg + 1) * 128],
                                        qT[:, h,
                                           qc * 512 + c0:(qc + 1) * 512],
                                        start=True, stop=True)
                                at = pba.tile([128, 2, 512], BF,
                                              name="at", tag="at")
                                if len(slots) == 2:
                                    nc.scalar.activation(
                                        out=at, in_=sc, func=EXP,
                                        scale=ALPHA)
                                else:
                                    jg, c0 = slots[0]
                                    nc.scalar.activation(
                                        out=at[:, 0, c0:512],
                                        in_=sc[:, 0, c0:512],
                                        func=EXP, scale=ALPHA)
                                    nc.vector.tensor_mul(
                                        at[:, 0, c0:c0 + 128],
                                        at[:, 0, c0:c0 + 128], mtri)
                                if ei == 0:
                                    nc.vector.tensor_copy(out=acc,
                                                          in_=at[:, 0, :])
                                else:
                                    for s, (jg, c0) in enumerate(slots):
                                        nc.vector.tensor_add(
                                            acc[:, c0:512],
                                            acc[:, c0:512],
                                            at[:, s, c0:512])
                                ats[ei] = (at, slots)

                            for ei in range(min(la, n)):
                                emit(ei)
                            finalize()
                            for ei in range(n):
                                if ei + la < n:
                                    emit(ei + la)
                                at, slots = ats.pop(ei)
                                for s, (jg, c0) in enumerate(slots):
                                    nc.tensor.matmul(
                                        ypsum[:, c0:512],
                                        vt[:, jg,
                                           h2 * 128:(h2 + 1) * 128],
                                        at[:, s, c0:512],
                                        start=(ei == 0 and s == 0),
                                        stop=(ei == n - 1 and
                                              s == len(slots) - 1))
                                c_step(cpace)
                            pending[0] = (ypsum, acc, h, yq, qc)
                        finalize()
                        c_step(200)  # drain any leftover out-proj steps
                        cg[0] = c_gen(yq, qc)
                        credit[0] = 0.0
                        nxt = 8 * (4 + 2 * (qc + 1))
                        cpace = DT * (NH + 1) / nxt
                    c_step(200)

            pbr.release()
            pac.release()
            pba.release()
            pbv.release()
    nc.compile()
    return nc


_NC_CACHE = None


def _get_program():
    global _NC_CACHE
    if _NC_CACHE is None:
        _NC_CACHE = build_program()
    return _NC_CACHE


def _host_inputs(x, w_qkv, w_o):
    inv = 1.0 / (ROPE_BASE ** (np.arange(0, HD, 2, dtype=np.float64) / HD))
    ang = np.arange(L, dtype=np.float64)[:, None] * inv[None, :]
    chalf = np.cos(ang).astype(BF16)                           # [L, 64]
    shalf = np.sin(ang).astype(BF16)
    p = np.arange(128)[:, None]
    f = np.arange(128)[None, :]
    mtri = (p <= f).astype(BF16)                               # [128, 128]
    ident = np.eye(128).astype(BF16)
    # per-head [even dims | odd dims] permutation for q/k rows
    perm = np.concatenate([np.arange(0, 128, 2), np.arange(1, 128, 2)])

    in_maps = []
    for c in range(8):
        b, g = c % 4, c // 4
        qr = w_qkv[g * DL:(g + 1) * DL]
        kr = w_qkv[D + g * DL:D + (g + 1) * DL]
        vr = w_qkv[2 * D + g * DL:2 * D + (g + 1) * DL]
        qp = qr.reshape(NH, HD, D)[:, perm, :].reshape(DL, D)
        kp = kr.reshape(NH, HD, D)[:, perm, :].reshape(DL, D)
        wqkvT = np.ascontiguousarray(
            np.concatenate([qp[:512], kp[:512], vr[:512],
                            vr[512:], qp[512:], kp[512:]], axis=0).T
        ).astype(BF16)
        in_maps.append({
            "xT": np.ascontiguousarray(x[b].T).astype(BF16),
            "wqkvT": wqkvT,
            "woT": np.ascontiguousarray(
                w_o[:, g * DL:(g + 1) * DL].T).astype(BF16),
            "chalf": chalf,
            "shalf": shalf,
            "mtri": mtri,
            "ident": ident,
        })
    return in_maps


def kernel(x, w_qkv, w_o, _trace=False):
    x = np.asarray(x, dtype=np.float32)
    w_qkv = np.asarray(w_qkv, dtype=np.float32)
    w_o = np.asarray(w_o, dtype=np.float32)
    nc = _get_program()
    in_maps = _host_inputs(x, w_qkv, w_o)
    res = run_bass_kernel_spmd(nc, in_maps, core_ids=list(range(8)),
                               trace=_trace)
    kernel.last_result = res
    parts = [np.asarray(r["outT"], dtype=np.float32) for r in res.results]
    out = np.empty((B, L, D), dtype=np.float32)
    for b in range(B):
        out[b] = (parts[b] + parts[b + 4]).T
    return out


# revision 19
# speedup vs baseline: 1.1447x; 1.0162x over previous
"""MHA (RoPE + causal softmax attention + out-proj) on 8 NeuronCores.

Sharding: DP4 x TP2. Core c: batch b = c % 4, head-group g = c // 4
(8 heads per core). Each core computes a transposed partial output
outT = (y_local @ w_o_slice^T)^T in [D, L] (bf16); host sums the two
head-group partials per batch and transposes back.

All matmuls bf16 x bf16 -> fp32 PSUM. Layout strategy (v2):
  Phase A: qkv natural layout [L, comps] via out = xT_tile.T @ w_chunk,
           d-outer accumulation over 6-tile L-groups so the PE starts as
           soon as the first (x, w) d-pair lands. q/k head-dims are
           pre-permuted host-side to [even|odd] halves so RoPE uses
           packed (unstrided) APs: one ScalarE copy PSUM->bf16, then 6
           DVE ops. Rotated q/k tiles are transposed on the PE
           (128x128 transpose-matmuls -> bf16 PSUM -> one DVE copy)
           into SBUF-resident qT/kT [hd, L] per head - no DRAM round
           trip, no DMA-xbar transposes. v is staged to DRAM (natural
           layout) via gpsimd-issued DMAs.
  Phase B: per head, scores computed transposed (k on partitions) from
           SBUF qT/kT; exp on ScalarE with the 1/sqrt(HD) scale fused.
           Causal: fully-masked k-tiles skipped, diagonal k-tiles
           truncated to the valid q-range (matmul cols 512/384/256/128),
           single [128,128] triangle mask. Softmax denominator via DVE
           bf16 accumulation of the exp tiles + ONE all-ones matmul per
           (head, q-chunk) instead of a ones-matmul per k-tile.
  Phase C: out-proj outT[e, q] = sum_d w_oT[d, e] * yT[d, q], bf16 out.
"""

import numpy as np
import ml_dtypes

import concourse.bass as bass
import concourse.tile as tile
import concourse.mybir as mybir
from concourse import bacc
from concourse.bass_utils import run_bass_kernel_spmd

BF16 = ml_dtypes.bfloat16
F32 = mybir.dt.float32
BF = mybir.dt.bfloat16

B, L, D, H, HD = 4, 2048, 2048, 16, 128
NH = 8                      # heads per core
DL = NH * HD                # 1024 local head dims
ROPE_BASE = 10000.0
ALPHA = float(HD) ** -0.5

LT = L // 128               # 16 L-tiles
DT = D // 128               # 16 D(contract)-tiles
NCH = 6                     # qkv chunks of 512 comps: q03,k03,v03,q47,k47,v47
QC = L // 512               # 4 q-chunks of 512
KT = L // 128               # 16 k-tiles
GROUPS = [(0, 6), (6, 12), (12, 16)]

EXP = mybir.ActivationFunctionType.Exp


CHUNKS = [("q", 0), ("k", 0), ("v", 0), ("v", 1), ("q", 1), ("k", 1)]


def _chunk_kind(c):
    # chunk order: q(heads0-3), k(0-3), v(0-3), v(4-7), q(4-7), k(4-7)
    return CHUNKS[c]


def build_program(phases="ABC", la=1):
    nc = bacc.Bacc("TRN2", target_bir_lowering=False, debug=False, num_devices=8)

    xT = nc.dram_tensor("xT", [D, L], BF, kind="ExternalInput").ap()
    wqkvT = nc.dram_tensor("wqkvT", [D, 3 * DL], BF, kind="ExternalInput").ap()
    woT = nc.dram_tensor("woT", [DL, D], BF, kind="ExternalInput").ap()
    chalf = nc.dram_tensor("chalf", [L, 64], BF, kind="ExternalInput").ap()
    shalf = nc.dram_tensor("shalf", [L, 64], BF, kind="ExternalInput").ap()
    mtri_d = nc.dram_tensor("mtri", [128, 128], BF, kind="ExternalInput").ap()
    ident_d = nc.dram_tensor("ident", [128, 128], BF, kind="ExternalInput").ap()
    outT = nc.dram_tensor("outT", [D, L], BF, kind="ExternalOutput").ap()

    # DRAM staging for v (natural layout), split per head-group so the
    # vt reloads for heads 0-3 have no dependency on the group-1 writes
    vnat0 = nc.dram_tensor("vnat0", [L, 512], BF, kind="Internal").ap()
    vnat1 = nc.dram_tensor("vnat1", [L, 512], BF, kind="Internal").ap()

    with tile.TileContext(nc) as tc:
        with tc.tile_pool(name="outer", bufs=1) as outer:
            # persistent: transposed rotated q/k per head [hd, L]
            qT = outer.tile([128, NH, L], BF, name="qT", tag="qT")
            kT = outer.tile([128, NH, L], BF, name="kT", tag="kT")
            c_sb = outer.tile([128, LT, 1, 64], BF, name="c_sb", tag="c_sb")
            s_sb = outer.tile([128, LT, 1, 64], BF, name="s_sb", tag="s_sb")
            ones128 = outer.tile([128, 128], BF, name="ones128", tag="oc")
            zeros = outer.tile([128, 512], BF, name="zeros", tag="zc")
            mtri = outer.tile([128, 128], BF, name="mtri", tag="mtri")
            idt = outer.tile([128, 128], BF, name="idt", tag="idt")
            nc.vector.memset(ones128, 1.0)
            nc.vector.memset(zeros, 0.0)
            if "B" not in phases:
                nc.vector.memset(qT, 0.0)
                nc.vector.memset(kT, 0.0)

            # small phase-B pools opened FIRST so their SBUF addresses do
            # not overlap the phase-A zone (no release dependency on the
            # phase-A drain)
            pbv = tc.alloc_tile_pool(name="pBv", bufs=1)
            pba = tc.alloc_tile_pool(name="pBa", bufs=3)
            pac = tc.alloc_tile_pool(name="pBc", bufs=2)
            pbr = tc.alloc_tile_pool(name="pBr", bufs=1)
            vts = [None] * 4

            # ---------------- Phase A: QKV + RoPE + transpose ----------------
            with tc.tile_pool(name="pA", bufs=1) as pa, \
                 tc.tile_pool(name="pAw", bufs=5) as paw, \
                 tc.tile_pool(name="pAt", bufs=1) as pat, \
                 tc.tile_pool(name="pAo", bufs=4) as pao, \
                 tc.tile_pool(name="pAr", bufs=2) as pro, \
                 tc.tile_pool(name="psA", bufs=1, space="PSUM") as psa, \
                 tc.tile_pool(name="psT", bufs=2, space="PSUM") as pst:
                # PE warm-up: a few dummy matmuls so the p-state ramp runs
                # during the initial DMA loads instead of on real work
                warm = psa.tile([128, 512], F32, name="pn", tag="pn0")
                for _ in range(8):
                    nc.tensor.matmul(warm, ones128, zeros, start=True,
                                     stop=True)
                # resident xT tiles [128, L] per D-tile; loaded in column
                # halves interleaved with chunk-0 weight d-tiles so the
                # d-outer accumulation can start almost immediately.
                xts = [pa.tile([128, L], BF, name=f"xt{d}", tag=f"xt{d}")
                       for d in range(DT)]

                def load_wch_q(c, quarter):
                    wh = paw.tile([128, 4, 512], BF, name="wch", tag="wch")
                    for dd in range(4):
                        d = quarter * 4 + dd
                        nc.sync.dma_start(
                            out=wh[:, dd, :],
                            in_=wqkvT[d * 128:(d + 1) * 128,
                                      c * 512:(c + 1) * 512])
                    return wh

                wh0 = [paw.tile([128, 4, 512], BF, name="wch", tag="wch")
                       for _ in range(4)]
                for d in range(DT):
                    nc.sync.dma_start(out=xts[d][:, 0:1024],
                                      in_=xT[d * 128:(d + 1) * 128, 0:1024])
                    nc.sync.dma_start(
                        out=wh0[d // 4][:, d % 4, :],
                        in_=wqkvT[d * 128:(d + 1) * 128, 0:512])
                nc.sync.dma_start(
                    out=c_sb,
                    in_=chalf.rearrange("(i p) g -> p i g", p=128))
                nc.sync.dma_start(
                    out=s_sb,
                    in_=shalf.rearrange("(i p) g -> p i g", p=128))
                nc.sync.dma_start(out=idt, in_=ident_d)
                nc.sync.dma_start(out=mtri, in_=mtri_d)
                for d in range(DT):
                    nc.sync.dma_start(out=xts[d][:, 1024:2048],
                                      in_=xT[d * 128:(d + 1) * 128, 1024:2048])

                for c in range(NCH if "A" in phases else 0):
                    kind, grp = _chunk_kind(c)
                    if c == 0:
                        whq = wh0
                    else:
                        whq = [load_wch_q(c, q) for q in range(4)]
                    for (lo, hi) in GROUPS:
                        pns = [psa.tile([128, 512], F32, name="pn", tag=f"pn{i - lo}")
                               for i in range(lo, hi)]
                        for d in range(DT):
                            for i in range(lo, hi):
                                nc.tensor.matmul(
                                    pns[i - lo],
                                    xts[d][:, i * 128:(i + 1) * 128],
                                    whq[d // 4][:, d % 4, :],
                                    start=(d == 0), stop=(d == DT - 1))
                        for i in range(lo, hi):
                            pn = pns[i - lo]
                            if kind == "v":
                                vo = pao.tile([128, 512], BF, name="vo", tag="vo")
                                nc.scalar.copy(out=vo, in_=pn)
                                vn = vnat0 if grp == 0 else vnat1
                                nc.gpsimd.dma_start(
                                    out=vn[i * 128:(i + 1) * 128, :],
                                    in_=vo)
                            else:
                                xb = pao.tile([128, 4, 128], BF, name="xb", tag="vo")
                                nc.scalar.copy(out=xb, in_=pn)
                                x1 = xb[:, :, 0:64]
                                x2 = xb[:, :, 64:128]
                                ct = c_sb[:, i].broadcast_to([128, 4, 64])
                                st = s_sb[:, i].broadcast_to([128, 4, 64])
                                t1 = pat.tile([128, 4, 64], BF, name="t1", tag="t1")
                                nc.vector.tensor_mul(t1, x1, ct)
                                t2 = pat.tile([128, 4, 64], BF, name="t2", tag="t2")
                                nc.vector.tensor_mul(t2, x2, st)
                                t3 = pat.tile([128, 4, 64], BF, name="t3", tag="t3")
                                nc.vector.tensor_mul(t3, x2, ct)
                                t4 = pat.tile([128, 4, 64], BF, name="t4", tag="t4")
                                nc.vector.tensor_mul(t4, x1, st)
                                ro = pro.tile([128, 4, 128], BF, name="ro", tag="ro")
                                nc.vector.tensor_sub(ro[:, :, 0:64], t1, t2)
                                nc.vector.tensor_add(ro[:, :, 64:128], t3, t4)
                                pt = pst.tile([128, 4, 128], BF, name="pt", tag="pt")
                                for hh in range(4):
                                    nc.tensor.transpose(
                                        pt[:, hh, :], ro[:, hh, :], idt)
                                dstT = qT if kind == "q" else kT
                                nc.vector.tensor_copy(
                                    out=dstT[:, grp * 4:(grp + 1) * 4,
                                             i * 128:(i + 1) * 128],
                                    in_=pt)
                    if kind == "v" and "B" in phases:
                        # this head-group's v fully staged: prefetch its vt
                        vn = vnat0 if grp == 0 else vnat1
                        for hp in (2 * grp, 2 * grp + 1):
                            vt = pbv.tile([128, KT, 256], BF,
                                          name=f"vt{hp}", tag=f"vt{hp}")
                            nc.sync.dma_start(
                                out=vt,
                                in_=vn[:, (hp % 2) * 256:(hp % 2) * 256 + 256]
                                    .rearrange("(j p) d -> p j d", p=128))
                            vts[hp] = vt

            # ---------------- Phase B+C: attention (qc-outer) with the
            # out-projection of the previous q-chunk interleaved into the
            # PE stream as filler for the exp-bound stretches ----------------
            with tc.tile_pool(name="pBw", bufs=1) as pbw, \
                 tc.tile_pool(name="pBy", bufs=2) as pby, \
                 tc.tile_pool(name="pCo", bufs=4) as pco:
                # phase-C weights: loaded early in B (after A frees)
                wos = []
                for dd in range(NH):
                    wo = pbw.tile([128, D], BF, name=f"wo{dd}", tag=f"wo{dd}")
                    nc.sync.dma_start(
                        out=wo, in_=woT[dd * 128:(dd + 1) * 128, :])
                    wos.append(wo)

                with tc.tile_pool(name="psS", bufs=2, space="PSUM") as pss, \
                     tc.tile_pool(name="psY", bufs=2, space="PSUM") as psy, \
                     tc.tile_pool(name="psC", bufs=2, space="PSUM") as psc:
                    # deferred per-(h, qc) epilogue so the denominator chain
                    # never blocks the PE FIFO
                    pending = [None]

                    def finalize():
                        if pending[0] is None:
                            return
                        ypsum_f, acc_f, h_f, yq_f, qc_f = pending[0]
                        pending[0] = None
                        if qc_f == 0:
                            # close the psum accumulation group on columns
                            # the truncated diagonal tiles never re-touched
                            nc.tensor.matmul(
                                ypsum_f[:, 0:384], ones128, zeros[:, 0:384],
                                start=False, stop=True)
                        dps = pss.tile([128, 2, 512], F32, name="dps", tag="sc")
                        nc.tensor.matmul(dps[:, 0, :], ones128, acc_f,
                                         start=True, stop=True)
                        rbs = pbr.tile([128, 512], BF, name="rbs", tag="rbs")
                        with nc.allow_low_precision("softmax recip bf16"):
                            nc.vector.reciprocal(out=rbs, in_=dps[:, 0, :])
                        nc.vector.tensor_mul(yq_f[:, h_f, :], ypsum_f, rbs)

                    def c_gen(yq, qcc):
                        # out-projection of q-chunk qcc, one PE matmul per
                        # yield step
                        for e in range(DT):
                            op = psc.tile([128, 512], F32, name="op", tag="op")
                            for dd in range(NH):
                                nc.tensor.matmul(
                                    op,
                                    wos[dd][:, e * 128:(e + 1) * 128],
                                    yq[:, dd, :],
                                    start=(dd == 0), stop=(dd == NH - 1))
                                yield
                            ot = pco.tile([128, 512], BF, name="ot", tag="ot")
                            nc.scalar.copy(out=ot, in_=op)
                            nc.sync.dma_start(
                                out=outT[e * 128:(e + 1) * 128,
                                         qcc * 512:(qcc + 1) * 512],
                                in_=ot)
                            yield

                    cg = [None]
                    credit = [0.0]

                    def c_step(k):
                        if cg[0] is None:
                            return
                        credit[0] += k
                        while credit[0] >= 1.0:
                            credit[0] -= 1.0
                            if next(cg[0], "done") == "done":
                                cg[0] = None
                                return

                    cpace = 0.0
                    for qc in range(QC if "B" in phases else 0):
                        yq = pby.tile([128, NH, 512], BF, name="yq", tag="yq")
                        for h in range(NH):
                            hp, h2 = h // 2, h % 2
                            vt = vts[hp]
                            # entries: two k-tiles per sc pair-tile.
                            # Diagonal tiles keep separate (truncated) exps;
                            # below-diagonal pairs share one wide exp to
                            # amortize ScalarE access latency.
                            entries = [[(4 * qc, 0, True),
                                        (4 * qc + 1, 128, True)],
                                       [(4 * qc + 2, 256, True),
                                        (4 * qc + 3, 384, True)]]
                            entries += [[(j, 0, False), (j + 1, 0, False)]
                                        for j in range(0, 4 * qc, 2)]
                            n = len(entries)
                            ypsum = psy.tile([128, 512], F32, name="ypsum",
                                             tag="yp")
                            acc = pac.tile([128, 512], BF, name="acc",
                                           tag="acc")
                            ats = {}

                            def emit(ei, h=h, qc=qc, entries=entries,
                                     acc=acc, ats=ats):
                                slots = entries[ei]
                                diag = slots[0][2]
                                sc = pss.tile([128, 2, 512], F32,
                                              name="sc", tag="sc")
                                for s, (jg, c0, _) in enumerate(slots):
                                    nc.tensor.matmul(
                                        sc[:, s, c0:512],
                                        kT[:, h, jg * 128:(jg + 1) * 128],
                                        qT[:, h,
                                           qc * 512 + c0:(qc + 1) * 512],
                                        start=True, stop=True)
                                at = pba.tile([128, 2, 512], BF,
                                              name="at", tag="at")
                                if not diag:
                                    nc.scalar.activation(
                                        out=at, in_=sc, func=EXP,
                                        scale=ALPHA)
                                else:
                                    for s, (jg, c0, _) in enumerate(slots):
                                        nc.scalar.activation(
                                            out=at[:, s, c0:512],
                                            in_=sc[:, s, c0:512],
                                            func=EXP, scale=ALPHA)
                                        nc.vector.tensor_mul(
                                            at[:, s, c0:c0 + 128],
                                            at[:, s, c0:c0 + 128], mtri)
                                if ei == 0:
                                    nc.vector.tensor_copy(out=acc,
                                                          in_=at[:, 0, :])
                                    s, (jg, c0, _) = 1, slots[1]
                                    nc.vector.tensor_add(
                                        acc[:, c0:512], acc[:, c0:512],
                                        at[:, 1, c0:512])
                                else:
                                    for s, (jg, c0, _) in enumerate(slots):
                                        nc.vector.tensor_add(
                                            acc[:, c0:512],
                                            acc[:, c0:512],
                                            at[:, s, c0:512])
                                ats[ei] = (at, slots)

                            for ei in range(min(la, n)):
                                emit(ei)
                            finalize()
                            for ei in range(n):
                                if ei + la < n:
                                    emit(ei + la)
                                at, slots = ats.pop(ei)
                                for s, (jg, c0, _) in enumerate(slots):
                                    nc.tensor.matmul(
                                        ypsum[:, c0:512],
                                        vt[:, jg,
                                           h2 * 128:(h2 + 1) * 128],
                                        at[:, s, c0:512],
                                        start=(ei == 0 and s == 0),
                                        stop=(ei == n - 1 and
                                              s == len(slots) - 1))
                                c_step(cpace)
                            pending[0] = (ypsum, acc, h, yq, qc)
                        finalize()
                        c_step(200)  # drain any leftover out-proj steps
                        cg[0] = c_gen(yq, qc)
                        credit[0] = 0.0
                        nxt = 8 * (2 + 2 * (qc + 1))
                        cpace = DT * (NH + 1) / nxt
                    c_step(200)

            pbr.release()
            pac.release()
            pba.release()
            pbv.release()
    nc.compile()
    return nc


_NC_CACHE = None


def _get_program():
    global _NC_CACHE
    if _NC_CACHE is None:
        _NC_CACHE = build_program()
    return _NC_CACHE


def _host_inputs(x, w_qkv, w_o):
    inv = 1.0 / (ROPE_BASE ** (np.arange(0, HD, 2, dtype=np.float64) / HD))
    ang = np.arange(L, dtype=np.float64)[:, None] * inv[None, :]
    chalf = np.cos(ang).astype(BF16)                           # [L, 64]
    shalf = np.sin(ang).astype(BF16)
    p = np.arange(128)[:, None]
    f = np.arange(128)[None, :]
    mtri = (p <= f).astype(BF16)                               # [128, 128]
    ident = np.eye(128).astype(BF16)
    # per-head [even dims | odd dims] permutation for q/k rows
    perm = np.concatenate([np.arange(0, 128, 2), np.arange(1, 128, 2)])

    in_maps = []
    for c in range(8):
        b, g = c % 4, c // 4
        qr = w_qkv[g * DL:(g + 1) * DL]
        kr = w_qkv[D + g * DL:D + (g + 1) * DL]
        vr = w_qkv[2 * D + g * DL:2 * D + (g + 1) * DL]
        qp = qr.reshape(NH, HD, D)[:, perm, :].reshape(DL, D)
        kp = kr.reshape(NH, HD, D)[:, perm, :].reshape(DL, D)
        wqkvT = np.ascontiguousarray(
            np.concatenate([qp[:512], kp[:512], vr[:512],
                            vr[512:], qp[512:], kp[512:]], axis=0).T
        ).astype(BF16)
        in_maps.append({
            "xT": np.ascontiguousarray(x[b].T).astype(BF16),
            "wqkvT": wqkvT,
            "woT": np.ascontiguousarray(
                w_o[:, g * DL:(g + 1) * DL].T).astype(BF16),
            "chalf": chalf,
            "shalf": shalf,
            "mtri": mtri,
            "ident": ident,
        })
    return in_maps


def kernel(x, w_qkv, w_o, _trace=False):
    x = np.asarray(x, dtype=np.float32)
    w_qkv = np.asarray(w_qkv, dtype=np.float32)
    w_o = np.asarray(w_o, dtype=np.float32)
    nc = _get_program()
    in_maps = _host_inputs(x, w_qkv, w_o)
    res = run_bass_kernel_spmd(nc, in_maps, core_ids=list(range(8)),
                               trace=_trace)
    kernel.last_result = res
    parts = [np.asarray(r["outT"], dtype=np.float32) for r in res.results]
    out = np.empty((B, L, D), dtype=np.float32)
    for b in range(B):
        out[b] = (parts[b] + parts[b + 4]).T
    return out


# revision 55
# speedup vs baseline: 1.1631x; 1.0161x over previous
"""MHA (RoPE + causal softmax attention + out-proj) on 8 NeuronCores.

Sharding: DP4 x TP2. Core c: batch b = c % 4, head-group g = c // 4
(8 heads per core). Each core computes a transposed partial output
outT = (y_local @ w_o_slice^T)^T in [D, L] (bf16); host sums the two
head-group partials per batch and transposes back.

All matmuls bf16 x bf16 -> fp32 PSUM; the kernel is tensor-engine bound,
so the design minimizes PE columns and keeps the PE stream dense:

  Phase A (QKV + RoPE + transpose): out = xT_tile.T @ w_chunk with
    d-outer accumulation over 6-tile L-groups, so the PE starts as soon
    as the first (x, w) d-pair lands (weights DMA'd as per-d quarter
    chunks; x in column halves; a few warm-up matmuls cover the initial
    DMA latency and the p-state ramp). q/k head-dims are host-permuted
    to [even|odd] halves so RoPE uses packed APs: one ScalarE
    PSUM->bf16 copy, then 6 DVE ops (cos/sin broadcast via 0-stride
    APs). Rotated q/k tiles are transposed on the PE (128x128
    transpose-matmuls -> bf16 PSUM -> one DVE copy) into SBUF-resident
    qT/kT [hd, L] - no DRAM round trip, no DMA-xbar transposes. v is
    staged to DRAM (split per head-group; gpsimd-issued DMAs) and
    prefetched back as SBUF vt tiles as soon as each group is staged.
    Chunk order q03,k03,v03,v47,q47,k47 so both v groups stage early.

  Phase B+C (attention, qc-outer, out-proj interleaved): per q-chunk,
    all 8 heads run scores/softmax/attn@V; the out-projection of the
    PREVIOUS q-chunk is fed into the PE stream one matmul at a time,
    paced evenly across the attention entries, which keeps the PE busy
    through the exp-bound stretches. Scores come out transposed (k on
    partitions) so exp tiles are directly the moving operand of attn@V.
    Causal handling: fully-masked k-tiles skipped; diagonal k-tiles
    truncated to their valid q-range (cols 512/384/256/128) with a
    single [128,128] triangle mask; two k-tiles share each PSUM
    pair-tile (below-diagonal pairs share one wide exp to amortize
    ScalarE access latency). Softmax denominator: DVE bf16 accumulation
    of the exp tiles + ONE all-ones matmul per (head, q-chunk); the
    denominator epilogue is deferred past the next chunk's first
    score matmuls so it never blocks the PE FIFO. Small phase-B pools
    are allocated outside the phase-A SBUF zone so nothing in B waits
    on the A drain. Output written bf16.
"""

import numpy as np
import ml_dtypes

import concourse.bass as bass
import concourse.tile as tile
import concourse.mybir as mybir
from concourse import bacc
from concourse.bass_utils import run_bass_kernel_spmd

BF16 = ml_dtypes.bfloat16
F32 = mybir.dt.float32
BF = mybir.dt.bfloat16

B, L, D, H, HD = 4, 2048, 2048, 16, 128
NH = 8                      # heads per core
DL = NH * HD                # 1024 local head dims
ROPE_BASE = 10000.0
ALPHA = float(HD) ** -0.5

LT = L // 128               # 16 L-tiles
DT = D // 128               # 16 D(contract)-tiles
NCH = 6                     # qkv chunks of 512 comps: q03,k03,v03,q47,k47,v47
QC = L // 512               # 4 q-chunks of 512
KT = L // 128               # 16 k-tiles
GROUPS = [(0, 6), (6, 12), (12, 16)]

EXP = mybir.ActivationFunctionType.Exp


CHUNKS = [("q", 0), ("k", 0), ("v", 0), ("v", 1), ("q", 1), ("k", 1)]


def _chunk_kind(c):
    # chunk order: q(heads0-3), k(0-3), v(0-3), v(4-7), q(4-7), k(4-7)
    return CHUNKS[c]


def build_program(phases="ABC", la=3):
    nc = bacc.Bacc("TRN2", target_bir_lowering=False, debug=False, num_devices=8)

    xT = nc.dram_tensor("xT", [D, L], BF, kind="ExternalInput").ap()
    wqkvT = nc.dram_tensor("wqkvT", [D, 3 * DL], BF, kind="ExternalInput").ap()
    woT = nc.dram_tensor("woT", [DL, D], BF, kind="ExternalInput").ap()
    chalf = nc.dram_tensor("chalf", [L, 64], BF, kind="ExternalInput").ap()
    shalf = nc.dram_tensor("shalf", [L, 64], BF, kind="ExternalInput").ap()
    mtri_d = nc.dram_tensor("mtri", [128, 128], BF, kind="ExternalInput").ap()
    ident_d = nc.dram_tensor("ident", [128, 128], BF, kind="ExternalInput").ap()
    outT = nc.dram_tensor("outT", [D, L], BF, kind="ExternalOutput").ap()

    # DRAM staging for v (natural layout), split per head-group so the
    # vt reloads for heads 0-3 have no dependency on the group-1 writes
    vnat0 = nc.dram_tensor("vnat0", [L, 512], BF, kind="Internal").ap()
    vnat1 = nc.dram_tensor("vnat1", [L, 512], BF, kind="Internal").ap()

    with tile.TileContext(nc) as tc:
        with tc.tile_pool(name="outer", bufs=1) as outer:
            # persistent: transposed rotated q/k per head [hd, L]
            qT = outer.tile([128, NH, L], BF, name="qT", tag="qT")
            kT = outer.tile([128, NH, L], BF, name="kT", tag="kT")
            c_sb = outer.tile([128, LT, 1, 64], BF, name="c_sb", tag="c_sb")
            s_sb = outer.tile([128, LT, 1, 64], BF, name="s_sb", tag="s_sb")
            ones128 = outer.tile([128, 128], BF, name="ones128", tag="oc")
            zeros = outer.tile([128, 384], BF, name="zeros", tag="zc")
            mtri = outer.tile([128, 128], BF, name="mtri", tag="mtri")
            idt = outer.tile([128, 128], BF, name="idt", tag="idt")
            nc.vector.memset(ones128, 1.0)
            nc.vector.memset(zeros, 0.0)
            if "B" not in phases:
                nc.vector.memset(qT, 0.0)
                nc.vector.memset(kT, 0.0)

            # small phase-B pools opened FIRST so their SBUF addresses do
            # not overlap the phase-A zone (no release dependency on the
            # phase-A drain)
            pbv = tc.alloc_tile_pool(name="pBv", bufs=1)
            pba = tc.alloc_tile_pool(name="pBa", bufs=5)
            pac = tc.alloc_tile_pool(name="pBc", bufs=3)
            pbr = tc.alloc_tile_pool(name="pBr", bufs=1)
            vts = [None] * 4

            # ---------------- Phase A: QKV + RoPE + transpose ----------------
            with tc.tile_pool(name="pA", bufs=1) as pa, \
                 tc.tile_pool(name="pAw", bufs=5) as paw, \
                 tc.tile_pool(name="pAt", bufs=1) as pat, \
                 tc.tile_pool(name="pAo", bufs=4) as pao, \
                 tc.tile_pool(name="pAr", bufs=2) as pro, \
                 tc.tile_pool(name="psA", bufs=1, space="PSUM") as psa, \
                 tc.tile_pool(name="psT", bufs=2, space="PSUM") as pst:
                # PE warm-up: a few dummy matmuls so the p-state ramp runs
                # during the initial DMA loads instead of on real work
                warm = psa.tile([128, 512], F32, name="pn", tag="pn0")
                for _ in range(8):
                    nc.tensor.matmul(warm[:, 0:384], ones128, zeros, start=True,
                                     stop=True)
                # resident xT tiles [128, L] per D-tile; loaded in column
                # halves interleaved with chunk-0 weight d-tiles so the
                # d-outer accumulation can start almost immediately.
                xts = [pa.tile([128, L], BF, name=f"xt{d}", tag=f"xt{d}")
                       for d in range(DT)]

                def load_wch_q(c, quarter):
                    wh = paw.tile([128, 4, 512], BF, name="wch", tag="wch")
                    for dd in range(4):
                        d = quarter * 4 + dd
                        nc.sync.dma_start(
                            out=wh[:, dd, :],
                            in_=wqkvT[d * 128:(d + 1) * 128,
                                      c * 512:(c + 1) * 512])
                    return wh

                wh0 = [paw.tile([128, 4, 512], BF, name="wch", tag="wch")
                       for _ in range(4)]
                for d in range(DT):
                    nc.sync.dma_start(out=xts[d][:, 0:1024],
                                      in_=xT[d * 128:(d + 1) * 128, 0:1024])
                    nc.sync.dma_start(
                        out=wh0[d // 4][:, d % 4, :],
                        in_=wqkvT[d * 128:(d + 1) * 128, 0:512])
                nc.sync.dma_start(
                    out=c_sb,
                    in_=chalf.rearrange("(i p) g -> p i g", p=128))
                nc.sync.dma_start(
                    out=s_sb,
                    in_=shalf.rearrange("(i p) g -> p i g", p=128))
                nc.sync.dma_start(out=idt, in_=ident_d)
                nc.sync.dma_start(out=mtri, in_=mtri_d)
                for d in range(DT):
                    nc.sync.dma_start(out=xts[d][:, 1024:2048],
                                      in_=xT[d * 128:(d + 1) * 128, 1024:2048])

                pn_off = [0]
                for c in range(NCH if "A" in phases else 0):
                    kind, grp = _chunk_kind(c)
                    if c == 0:
                        whq = wh0
                    else:
                        whq = [load_wch_q(c, q) for q in range(4)]
                    groups = GROUPS if c < NCH - 1 else \
                        [(0, 6), (6, 12), (12, 14), (14, 15), (15, 16)]
                    for gi, (lo, hi) in enumerate(groups):
                        pns = [psa.tile([128, 512], F32, name="pn",
                                        tag=f"pn{(i - lo + 2 * gi) % 6}")
                               for i in range(lo, hi)]
                        for d in range(DT):
                            for i in range(lo, hi):
                                nc.tensor.matmul(
                                    pns[i - lo],
                                    xts[d][:, i * 128:(i + 1) * 128],
                                    whq[d // 4][:, d % 4, :],
                                    start=(d == 0), stop=(d == DT - 1))
                        for i in range(lo, hi):
                            pn = pns[i - lo]
                            if kind == "v":
                                vo = pao.tile([128, 512], BF, name="vo", tag="vo")
                                nc.scalar.copy(out=vo, in_=pn)
                                vn = vnat0 if grp == 0 else vnat1
                                nc.gpsimd.dma_start(
                                    out=vn[i * 128:(i + 1) * 128, :],
                                    in_=vo)
                            else:
                                xb = pao.tile([128, 4, 128], BF, name="xb", tag="vo")
                                nc.scalar.copy(out=xb, in_=pn)
                                x1 = xb[:, :, 0:64]
                                x2 = xb[:, :, 64:128]
                                ct = c_sb[:, i].broadcast_to([128, 4, 64])
                                st = s_sb[:, i].broadcast_to([128, 4, 64])
                                t1 = pat.tile([128, 4, 64], BF, name="t1", tag="t1")
                                nc.vector.tensor_mul(t1, x1, ct)
                                t2 = pat.tile([128, 4, 64], BF, name="t2", tag="t2")
                                nc.vector.tensor_mul(t2, x2, st)
                                t3 = pat.tile([128, 4, 64], BF, name="t3", tag="t3")
                                nc.vector.tensor_mul(t3, x2, ct)
                                t4 = pat.tile([128, 4, 64], BF, name="t4", tag="t4")
                                nc.vector.tensor_mul(t4, x1, st)
                                ro = pro.tile([128, 4, 128], BF, name="ro", tag="ro")
                                nc.vector.tensor_sub(ro[:, :, 0:64], t1, t2)
                                nc.vector.tensor_add(ro[:, :, 64:128], t3, t4)
                                pt = pst.tile([128, 4, 128], BF, name="pt", tag="pt")
                                for hh in range(4):
                                    nc.tensor.transpose(
                                        pt[:, hh, :], ro[:, hh, :], idt)
                                dstT = qT if kind == "q" else kT
                                nc.vector.tensor_copy(
                                    out=dstT[:, grp * 4:(grp + 1) * 4,
                                             i * 128:(i + 1) * 128],
                                    in_=pt)
                    if kind == "v" and "B" in phases:
                        # this head-group's v fully staged: prefetch its vt
                        vn = vnat0 if grp == 0 else vnat1
                        for hp in (2 * grp, 2 * grp + 1):
                            vt = pbv.tile([128, KT, 256], BF,
                                          name=f"vt{hp}", tag=f"vt{hp}")
                            nc.sync.dma_start(
                                out=vt,
                                in_=vn[:, (hp % 2) * 256:(hp % 2) * 256 + 256]
                                    .rearrange("(j p) d -> p j d", p=128))
                            vts[hp] = vt

            # ---------------- Phase B+C: attention (qc-outer) with the
            # out-projection of the previous q-chunk interleaved into the
            # PE stream as filler for the exp-bound stretches ----------------
            with tc.tile_pool(name="pBw", bufs=1) as pbw, \
                 tc.tile_pool(name="pBy", bufs=2) as pby, \
                 tc.tile_pool(name="pCo", bufs=4) as pco:
                # phase-C weights: loaded early in B (after A frees)
                wos = []
                for dd in range(NH):
                    wo = pbw.tile([128, D], BF, name=f"wo{dd}", tag=f"wo{dd}")
                    nc.sync.dma_start(
                        out=wo, in_=woT[dd * 128:(dd + 1) * 128, :])
                    wos.append(wo)

                with tc.tile_pool(name="psS", bufs=2, space="PSUM") as pss, \
                     tc.tile_pool(name="psY", bufs=2, space="PSUM") as psy, \
                     tc.tile_pool(name="psC", bufs=2, space="PSUM") as psc:
                    # deferred per-(h, qc) epilogue so the denominator chain
                    # never blocks the PE FIFO
                    pending = [None]

                    def finalize():
                        if pending[0] is None:
                            return
                        ypsum_f, acc_f, h_f, yq_f, qc_f = pending[0]
                        pending[0] = None
                        if qc_f == 0:
                            # close the psum accumulation group on columns
                            # the truncated diagonal tiles never re-touched
                            nc.tensor.matmul(
                                ypsum_f[:, 0:384], ones128, zeros[:, 0:384],
                                start=False, stop=True)
                        dps = pss.tile([128, 2, 512], F32, name="dps", tag="sc")
                        nc.tensor.matmul(dps[:, 0, :], ones128, acc_f,
                                         start=True, stop=True)
                        rbs = pbr.tile([128, 512], BF, name="rbs", tag="rbs")
                        with nc.allow_low_precision("softmax recip bf16"):
                            nc.vector.reciprocal(out=rbs, in_=dps[:, 0, :])
                        nc.vector.tensor_mul(yq_f[:, h_f, :], ypsum_f, rbs)

                    def c_gen(yq, qcc, e_start=0):
                        # out-projection of q-chunk qcc, one PE matmul per
                        # yield step
                        for e in range(e_start, DT):
                            op = psc.tile([128, 512], F32, name="op", tag="op")
                            for dd in range(NH):
                                nc.tensor.matmul(
                                    op,
                                    wos[dd][:, e * 128:(e + 1) * 128],
                                    yq[:, dd, :],
                                    start=(dd == 0), stop=(dd == NH - 1))
                                yield
                            ot = pco.tile([128, 512], BF, name="ot", tag="ot")
                            nc.scalar.copy(out=ot, in_=op)
                            nc.sync.dma_start(
                                out=outT[e * 128:(e + 1) * 128,
                                         qcc * 512:(qcc + 1) * 512],
                                in_=ot)
                            yield

                    cg = [None]
                    credit = [0.0]

                    def c_step(k):
                        if cg[0] is None:
                            return
                        credit[0] += k
                        while credit[0] >= 1.0:
                            credit[0] -= 1.0
                            if next(cg[0], "done") == "done":
                                cg[0] = None
                                return

                    cpace = 0.0
                    for qc in range(QC if "B" in phases else 0):
                        yq = pby.tile([128, NH, 512], BF, name="yq", tag="yq")
                        # qc=0 has no out-proj filler yet; accumulate its
                        # first two e-blocks incrementally as heads finalize,
                        # using the otherwise-idle psC banks as PE filler
                        cops = [psc.tile([128, 512], F32, name="op", tag="op")
                                for _ in range(2)] if qc == 0 else None

                        def cinc():
                            if cops is None or pending[0] is None:
                                finalize()
                                return
                            hf, yqf = pending[0][2], pending[0][3]
                            finalize()
                            for eidx in range(2):
                                nc.tensor.matmul(
                                    cops[eidx],
                                    wos[hf][:, eidx * 128:(eidx + 1) * 128],
                                    yqf[:, hf, :],
                                    start=(hf == 0), stop=(hf == NH - 1))

                        for h in range(NH):
                            hp, h2 = h // 2, h % 2
                            vt = vts[hp]
                            # entries: two k-tiles per sc pair-tile.
                            # Diagonal tiles keep separate (truncated) exps;
                            # below-diagonal pairs share one wide exp to
                            # amortize ScalarE access latency.
                            entries = [[(4 * qc, 0, True),
                                        (4 * qc + 1, 128, True)],
                                       [(4 * qc + 2, 256, True),
                                        (4 * qc + 3, 384, True)]]
                            entries += [[(j, 0, False), (j + 1, 0, False)]
                                        for j in range(0, 4 * qc, 2)]
                            n = len(entries)
                            ypsum = psy.tile([128, 512], F32, name="ypsum",
                                             tag="yp")
                            acc = pac.tile([128, 512], BF, name="acc",
                                           tag="acc")
                            ats = {}

                            def emit(ei, h=h, qc=qc, entries=entries,
                                     acc=acc, ats=ats):
                                slots = entries[ei]
                                diag = slots[0][2]
                                sc = pss.tile([128, 2, 512], F32,
                                              name="sc", tag="sc")
                                for s, (jg, c0, _) in enumerate(slots):
                                    nc.tensor.matmul(
                                        sc[:, s, c0:512],
                                        kT[:, h, jg * 128:(jg + 1) * 128],
                                        qT[:, h,
                                           qc * 512 + c0:(qc + 1) * 512],
                                        start=True, stop=True)
                                at = pba.tile([128, 2, 512], BF,
                                              name="at", tag="at")
                                if not diag:
                                    nc.scalar.activation(
                                        out=at, in_=sc, func=EXP,
                                        scale=ALPHA)
                                else:
                                    for s, (jg, c0, _) in enumerate(slots):
                                        nc.scalar.activation(
                                            out=at[:, s, c0:512],
                                            in_=sc[:, s, c0:512],
                                            func=EXP, scale=ALPHA)
                                        nc.vector.tensor_mul(
                                            at[:, s, c0:c0 + 128],
                                            at[:, s, c0:c0 + 128], mtri)
                                if ei == 0:
                                    nc.vector.tensor_copy(out=acc,
                                                          in_=at[:, 0, :])
                                    s, (jg, c0, _) = 1, slots[1]
                                    nc.vector.tensor_add(
                                        acc[:, c0:512], acc[:, c0:512],
                                        at[:, 1, c0:512])
                                else:
                                    for s, (jg, c0, _) in enumerate(slots):
                                        nc.vector.tensor_add(
                                            acc[:, c0:512],
                                            acc[:, c0:512],
                                            at[:, s, c0:512])
                                ats[ei] = (at, slots)

                            for ei in range(min(la, n)):
                                emit(ei)
                            cinc()
                            for ei in range(n):
                                if ei + la < n:
                                    emit(ei + la)
                                at, slots = ats.pop(ei)
                                for s, (jg, c0, _) in enumerate(slots):
                                    nc.tensor.matmul(
                                        ypsum[:, c0:512],
                                        vt[:, jg,
                                           h2 * 128:(h2 + 1) * 128],
                                        at[:, s, c0:512],
                                        start=(ei == 0 and s == 0),
                                        stop=(ei == n - 1 and
                                              s == len(slots) - 1))
                                c_step(cpace)
                            pending[0] = (ypsum, acc, h, yq, qc)
                        cinc()
                        if cops is not None:
                            for eidx in range(2):
                                ot = pco.tile([128, 512], BF, name="ot",
                                              tag="ot")
                                nc.scalar.copy(out=ot, in_=cops[eidx])
                                nc.sync.dma_start(
                                    out=outT[eidx * 128:(eidx + 1) * 128,
                                             0:512],
                                    in_=ot)
                        c_step(200)  # drain any leftover out-proj steps
                        es = 2 if qc == 0 else 0
                        cg[0] = c_gen(yq, qc, es)
                        credit[0] = 0.0
                        nxt = 8 * (2 + 2 * (qc + 1))
                        reserve = 0 if qc == QC - 2 else 0
                        cpace = ((DT - es) * (NH + 1) - reserve) / nxt
                    c_step(200)

            pbr.release()
            pac.release()
            pba.release()
            pbv.release()
    nc.compile()
    return nc


_NC_CACHE = None


def _get_program():
    global _NC_CACHE
    if _NC_CACHE is None:
        _NC_CACHE = build_program()
    return _NC_CACHE


def _host_inputs(x, w_qkv, w_o):
    inv = 1.0 / (ROPE_BASE ** (np.arange(0, HD, 2, dtype=np.float64) / HD))
    ang = np.arange(L, dtype=np.float64)[:, None] * inv[None, :]
    chalf = np.cos(ang).astype(BF16)                           # [L, 64]
    shalf = np.sin(ang).astype(BF16)
    p = np.arange(128)[:, None]
    f = np.arange(128)[None, :]
    mtri = (p <= f).astype(BF16)                               # [128, 128]
    ident = np.eye(128).astype(BF16)
    # per-head [even dims | odd dims] permutation for q/k rows
    perm = np.concatenate([np.arange(0, 128, 2), np.arange(1, 128, 2)])

    in_maps = []
    for c in range(8):
        b, g = c % 4, c // 4
        qr = w_qkv[g * DL:(g + 1) * DL]
        kr = w_qkv[D + g * DL:D + (g + 1) * DL]
        vr = w_qkv[2 * D + g * DL:2 * D + (g + 1) * DL]
        qp = qr.reshape(NH, HD, D)[:, perm, :].reshape(DL, D)
        kp = kr.reshape(NH, HD, D)[:, perm, :].reshape(DL, D)
        wqkvT = np.ascontiguousarray(
            np.concatenate([qp[:512], kp[:512], vr[:512],
                            vr[512:], qp[512:], kp[512:]], axis=0).T
        ).astype(BF16)
        in_maps.append({
            "xT": np.ascontiguousarray(x[b].T).astype(BF16),
            "wqkvT": wqkvT,
            "woT": np.ascontiguousarray(
                w_o[:, g * DL:(g + 1) * DL].T).astype(BF16),
            "chalf": chalf,
            "shalf": shalf,
            "mtri": mtri,
            "ident": ident,
        })
    return in_maps


def kernel(x, w_qkv, w_o, _trace=False):
    x = np.asarray(x, dtype=np.float32)
    w_qkv = np.asarray(w_qkv, dtype=np.float32)
    w_o = np.asarray(w_o, dtype=np.float32)
    nc = _get_program()
    in_maps = _host_inputs(x, w_qkv, w_o)
    res = run_bass_kernel_spmd(nc, in_maps, core_ids=list(range(8)),
                               trace=_trace)
    kernel.last_result = res
    parts = [np.asarray(r["outT"], dtype=np.float32) for r in res.results]
    out = np.empty((B, L, D), dtype=np.float32)
    for b in range(B):
        out[b] = (parts[b] + parts[b + 4]).T
    return out


# revision 57
# speedup vs baseline: 1.1656x; 1.0022x over previous
"""MHA (RoPE + causal softmax attention + out-proj) on 8 NeuronCores.

Sharding: DP4 x TP2. Core c: batch b = c % 4, head-group g = c // 4
(8 heads per core). Each core computes a transposed partial output
outT = (y_local @ w_o_slice^T)^T in [D, L] (bf16); host sums the two
head-group partials per batch and transposes back.

All matmuls bf16 x bf16 -> fp32 PSUM; the kernel is tensor-engine bound,
so the design minimizes PE columns and keeps the PE stream dense:

  Phase A (QKV + RoPE + transpose): out = xT_tile.T @ w_chunk with
    d-outer accumulation over 6-tile L-groups, so the PE starts as soon
    as the first (x, w) d-pair lands (weights DMA'd as per-d quarter
    chunks; x in column halves; a few warm-up matmuls cover the initial
    DMA latency and the p-state ramp). q/k head-dims are host-permuted
    to [even|odd] halves so RoPE uses packed APs: one ScalarE
    PSUM->bf16 copy, then 6 DVE ops (cos/sin broadcast via 0-stride
    APs). Rotated q/k tiles are transposed on the PE (128x128
    transpose-matmuls -> bf16 PSUM -> one DVE copy) into SBUF-resident
    qT/kT [hd, L] - no DRAM round trip, no DMA-xbar transposes. v is
    staged to DRAM (split per head-group; gpsimd-issued DMAs) and
    prefetched back as SBUF vt tiles as soon as each group is staged.
    Chunk order q03,k03,v03,v47,q47,k47 so both v groups stage early.

  Phase B+C (attention, qc-outer, out-proj interleaved): per q-chunk,
    all 8 heads run scores/softmax/attn@V; the out-projection of the
    PREVIOUS q-chunk is fed into the PE stream one matmul at a time,
    paced evenly across the attention entries, which keeps the PE busy
    through the exp-bound stretches. Scores come out transposed (k on
    partitions) so exp tiles are directly the moving operand of attn@V.
    Causal handling: fully-masked k-tiles skipped; diagonal k-tiles
    truncated to their valid q-range (cols 512/384/256/128) with a
    single [128,128] triangle mask; two k-tiles share each PSUM
    pair-tile (below-diagonal pairs share one wide exp to amortize
    ScalarE access latency). Softmax denominator: DVE bf16 accumulation
    of the exp tiles + ONE all-ones matmul per (head, q-chunk); the
    denominator epilogue is deferred past the next chunk's first
    score matmuls so it never blocks the PE FIFO. Small phase-B pools
    are allocated outside the phase-A SBUF zone so nothing in B waits
    on the A drain. Output written bf16.
"""

import numpy as np
import ml_dtypes

import concourse.bass as bass
import concourse.tile as tile
import concourse.mybir as mybir
from concourse import bacc
from concourse.bass_utils import run_bass_kernel_spmd

BF16 = ml_dtypes.bfloat16
F32 = mybir.dt.float32
BF = mybir.dt.bfloat16

B, L, D, H, HD = 4, 2048, 2048, 16, 128
NH = 8                      # heads per core
DL = NH * HD                # 1024 local head dims
ROPE_BASE = 10000.0
ALPHA = float(HD) ** -0.5

LT = L // 128               # 16 L-tiles
DT = D // 128               # 16 D(contract)-tiles
NCH = 6                     # qkv chunks of 512 comps: q03,k03,v03,q47,k47,v47
QC = L // 512               # 4 q-chunks of 512
KT = L // 128               # 16 k-tiles
GROUPS = [(0, 6), (6, 12), (12, 16)]

EXP = mybir.ActivationFunctionType.Exp


CHUNKS = [("q", 0), ("k", 0), ("v", 0), ("v", 1), ("q", 1), ("k", 1)]


def _chunk_kind(c):
    # chunk order: q(heads0-3), k(0-3), v(0-3), v(4-7), q(4-7), k(4-7)
    return CHUNKS[c]


def build_program(phases="ABC", la=3):
    nc = bacc.Bacc("TRN2", target_bir_lowering=False, debug=False, num_devices=8)

    xT = nc.dram_tensor("xT", [D, L], BF, kind="ExternalInput").ap()
    wqkvT = nc.dram_tensor("wqkvT", [D, 3 * DL], BF, kind="ExternalInput").ap()
    woT = nc.dram_tensor("woT", [DL, D], BF, kind="ExternalInput").ap()
    chalf = nc.dram_tensor("chalf", [L, 64], BF, kind="ExternalInput").ap()
    shalf = nc.dram_tensor("shalf", [L, 64], BF, kind="ExternalInput").ap()
    mtri_d = nc.dram_tensor("mtri", [128, 128], BF, kind="ExternalInput").ap()
    ident_d = nc.dram_tensor("ident", [128, 128], BF, kind="ExternalInput").ap()
    outT = nc.dram_tensor("outT", [D, L], BF, kind="ExternalOutput").ap()

    # DRAM staging for v (natural layout), split per head-group so the
    # vt reloads for heads 0-3 have no dependency on the group-1 writes
    vnat0 = nc.dram_tensor("vnat0", [L, 512], BF, kind="Internal").ap()
    vnat1 = nc.dram_tensor("vnat1", [L, 512], BF, kind="Internal").ap()

    with tile.TileContext(nc) as tc:
        with tc.tile_pool(name="outer", bufs=1) as outer:
            # persistent: transposed rotated q/k per head [hd, L]
            qT = outer.tile([128, NH, L], BF, name="qT", tag="qT")
            kT = outer.tile([128, NH, L], BF, name="kT", tag="kT")
            c_sb = outer.tile([128, LT, 1, 64], BF, name="c_sb", tag="c_sb")
            s_sb = outer.tile([128, LT, 1, 64], BF, name="s_sb", tag="s_sb")
            ones128 = outer.tile([128, 128], BF, name="ones128", tag="oc")
            zeros = outer.tile([128, 384], BF, name="zeros", tag="zc")
            mtri = outer.tile([128, 128], BF, name="mtri", tag="mtri")
            idt = outer.tile([128, 128], BF, name="idt", tag="idt")
            nc.vector.memset(ones128, 1.0)
            nc.vector.memset(zeros, 0.0)
            if "B" not in phases:
                nc.vector.memset(qT, 0.0)
                nc.vector.memset(kT, 0.0)

            # small phase-B pools opened FIRST so their SBUF addresses do
            # not overlap the phase-A zone (no release dependency on the
            # phase-A drain)
            pbv = tc.alloc_tile_pool(name="pBv", bufs=1)
            pba = tc.alloc_tile_pool(name="pBa", bufs=5)
            pac = tc.alloc_tile_pool(name="pBc", bufs=3)
            pbr = tc.alloc_tile_pool(name="pBr", bufs=1)
            vts = [None] * 4

            # ---------------- Phase A: QKV + RoPE + transpose ----------------
            with tc.tile_pool(name="pA", bufs=1) as pa, \
                 tc.tile_pool(name="pAw", bufs=5) as paw, \
                 tc.tile_pool(name="pAt", bufs=1) as pat, \
                 tc.tile_pool(name="pAo", bufs=4) as pao, \
                 tc.tile_pool(name="pAr", bufs=2) as pro, \
                 tc.tile_pool(name="psA", bufs=1, space="PSUM") as psa, \
                 tc.tile_pool(name="psT", bufs=2, space="PSUM") as pst:
                # PE warm-up: a few dummy matmuls so the p-state ramp runs
                # during the initial DMA loads instead of on real work
                warm = psa.tile([128, 512], F32, name="pn", tag="pn0")
                for _ in range(8):
                    nc.tensor.matmul(warm[:, 0:384], ones128, zeros, start=True,
                                     stop=True)
                # resident xT tiles [128, L] per D-tile; loaded in column
                # halves interleaved with chunk-0 weight d-tiles so the
                # d-outer accumulation can start almost immediately.
                xts = [pa.tile([128, L], BF, name=f"xt{d}", tag=f"xt{d}")
                       for d in range(DT)]

                def load_wch_q(c, quarter):
                    wh = paw.tile([128, 4, 512], BF, name="wch", tag="wch")
                    for dd in range(4):
                        d = quarter * 4 + dd
                        nc.sync.dma_start(
                            out=wh[:, dd, :],
                            in_=wqkvT[d * 128:(d + 1) * 128,
                                      c * 512:(c + 1) * 512])
                    return wh

                wh0 = [paw.tile([128, 4, 512], BF, name="wch", tag="wch")
                       for _ in range(4)]
                for d in range(DT):
                    nc.sync.dma_start(out=xts[d][:, 0:1024],
                                      in_=xT[d * 128:(d + 1) * 128, 0:1024])
                    nc.sync.dma_start(
                        out=wh0[d // 4][:, d % 4, :],
                        in_=wqkvT[d * 128:(d + 1) * 128, 0:512])
                nc.sync.dma_start(
                    out=c_sb,
                    in_=chalf.rearrange("(i p) g -> p i g", p=128))
                nc.sync.dma_start(
                    out=s_sb,
                    in_=shalf.rearrange("(i p) g -> p i g", p=128))
                nc.sync.dma_start(out=idt, in_=ident_d)
                nc.sync.dma_start(out=mtri, in_=mtri_d)
                for d in range(DT):
                    nc.sync.dma_start(out=xts[d][:, 1024:2048],
                                      in_=xT[d * 128:(d + 1) * 128, 1024:2048])

                pn_off = [0]
                for c in range(NCH if "A" in phases else 0):
                    kind, grp = _chunk_kind(c)
                    if c == 0:
                        whq = wh0
                    else:
                        whq = [load_wch_q(c, q) for q in range(4)]
                    groups = GROUPS if c < NCH - 1 else \
                        [(0, 6), (6, 12), (12, 14), (14, 15), (15, 16)]
                    for gi, (lo, hi) in enumerate(groups):
                        pns = [psa.tile([128, 512], F32, name="pn",
                                        tag=f"pn{(i - lo + 2 * gi) % 6}")
                               for i in range(lo, hi)]
                        for d in range(DT):
                            for i in range(lo, hi):
                                nc.tensor.matmul(
                                    pns[i - lo],
                                    xts[d][:, i * 128:(i + 1) * 128],
                                    whq[d // 4][:, d % 4, :],
                                    start=(d == 0), stop=(d == DT - 1))
                        for i in range(lo, hi):
                            pn = pns[i - lo]
                            if kind == "v":
                                vo = pao.tile([128, 512], BF, name="vo", tag="vo")
                                nc.scalar.copy(out=vo, in_=pn)
                                vn = vnat0 if grp == 0 else vnat1
                                nc.gpsimd.dma_start(
                                    out=vn[i * 128:(i + 1) * 128, :],
                                    in_=vo)
                            else:
                                xb = pao.tile([128, 4, 128], BF, name="xb", tag="vo")
                                nc.scalar.copy(out=xb, in_=pn)
                                x1 = xb[:, :, 0:64]
                                x2 = xb[:, :, 64:128]
                                ct = c_sb[:, i].broadcast_to([128, 4, 64])
                                st = s_sb[:, i].broadcast_to([128, 4, 64])
                                t1 = pat.tile([128, 4, 64], BF, name="t1", tag="t1")
                                nc.vector.tensor_mul(t1, x1, ct)
                                t2 = pat.tile([128, 4, 64], BF, name="t2", tag="t2")
                                nc.vector.tensor_mul(t2, x2, st)
                                t3 = pat.tile([128, 4, 64], BF, name="t3", tag="t3")
                                nc.vector.tensor_mul(t3, x2, ct)
                                t4 = pat.tile([128, 4, 64], BF, name="t4", tag="t4")
                                nc.vector.tensor_mul(t4, x1, st)
                                ro = pro.tile([128, 4, 128], BF, name="ro", tag="ro")
                                nc.vector.tensor_sub(ro[:, :, 0:64], t1, t2)
                                nc.vector.tensor_add(ro[:, :, 64:128], t3, t4)
                                pt = pst.tile([128, 4, 128], BF, name="pt", tag="pt")
                                for hh in range(4):
                                    nc.tensor.transpose(
                                        pt[:, hh, :], ro[:, hh, :], idt)
                                dstT = qT if kind == "q" else kT
                                nc.vector.tensor_copy(
                                    out=dstT[:, grp * 4:(grp + 1) * 4,
                                             i * 128:(i + 1) * 128],
                                    in_=pt)
                    if kind == "v" and "B" in phases:
                        # this head-group's v fully staged: prefetch its vt
                        vn = vnat0 if grp == 0 else vnat1
                        for hp in (2 * grp, 2 * grp + 1):
                            vt = pbv.tile([128, KT, 256], BF,
                                          name=f"vt{hp}", tag=f"vt{hp}")
                            nc.sync.dma_start(
                                out=vt,
                                in_=vn[:, (hp % 2) * 256:(hp % 2) * 256 + 256]
                                    .rearrange("(j p) d -> p j d", p=128))
                            vts[hp] = vt

            # ---------------- Phase B+C: attention (qc-outer) with the
            # out-projection of the previous q-chunk interleaved into the
            # PE stream as filler for the exp-bound stretches ----------------
            with tc.tile_pool(name="pBw", bufs=1) as pbw, \
                 tc.tile_pool(name="pBy", bufs=2) as pby, \
                 tc.tile_pool(name="pCo", bufs=4) as pco:
                # phase-C weights: loaded early in B (after A frees)
                wos = []
                for dd in range(NH):
                    wo = pbw.tile([128, D], BF, name=f"wo{dd}", tag=f"wo{dd}")
                    nc.sync.dma_start(
                        out=wo, in_=woT[dd * 128:(dd + 1) * 128, :])
                    wos.append(wo)

                with tc.tile_pool(name="psS", bufs=2, space="PSUM") as pss, \
                     tc.tile_pool(name="psY", bufs=2, space="PSUM") as psy, \
                     tc.tile_pool(name="psC", bufs=2, space="PSUM") as psc:
                    # deferred per-(h, qc) epilogue so the denominator chain
                    # never blocks the PE FIFO
                    pending = [None]

                    def finalize():
                        if pending[0] is None:
                            return
                        ypsum_f, acc_f, h_f, yq_f, qc_f = pending[0]
                        pending[0] = None
                        if qc_f == 0:
                            # close the psum accumulation group on columns
                            # the truncated diagonal tiles never re-touched
                            nc.tensor.matmul(
                                ypsum_f[:, 0:384], ones128, zeros[:, 0:384],
                                start=False, stop=True)
                        dps = pss.tile([128, 2, 512], F32, name="dps", tag="sc")
                        nc.tensor.matmul(dps[:, 0, :], ones128, acc_f,
                                         start=True, stop=True)
                        rbs = pbr.tile([128, 512], BF, name="rbs", tag="rbs")
                        with nc.allow_low_precision("softmax recip bf16"):
                            nc.vector.reciprocal(out=rbs, in_=dps[:, 0, :])
                        nc.vector.tensor_mul(yq_f[:, h_f, :], ypsum_f, rbs)

                    def c_gen(yq, qcc, e_start=0):
                        # out-projection of q-chunk qcc, one PE matmul per
                        # yield step
                        for e in range(e_start, DT):
                            op = psc.tile([128, 512], F32, name="op", tag="op")
                            for dd in range(NH):
                                nc.tensor.matmul(
                                    op,
                                    wos[dd][:, e * 128:(e + 1) * 128],
                                    yq[:, dd, :],
                                    start=(dd == 0), stop=(dd == NH - 1))
                                yield
                            ot = pco.tile([128, 512], BF, name="ot", tag="ot")
                            nc.scalar.copy(out=ot, in_=op)
                            nc.sync.dma_start(
                                out=outT[e * 128:(e + 1) * 128,
                                         qcc * 512:(qcc + 1) * 512],
                                in_=ot)
                            yield

                    cg = [None]
                    credit = [0.0]

                    def c_step(k):
                        if cg[0] is None:
                            return
                        credit[0] += k
                        while credit[0] >= 1.0:
                            credit[0] -= 1.0
                            if next(cg[0], "done") == "done":
                                cg[0] = None
                                return

                    cpace = 0.0
                    for qc in range(QC if "B" in phases else 0):
                        yq = pby.tile([128, NH, 512], BF, name="yq", tag="yq")
                        # qc=0 has no out-proj filler yet; accumulate its
                        # first two e-blocks incrementally as heads finalize,
                        # using the otherwise-idle psC banks as PE filler
                        cops = [psc.tile([128, 512], F32, name="op", tag="op")
                                for _ in range(2)] if qc == 0 else None

                        def cinc():
                            if cops is None or pending[0] is None:
                                finalize()
                                return
                            hf, yqf = pending[0][2], pending[0][3]
                            finalize()
                            for eidx in range(2):
                                nc.tensor.matmul(
                                    cops[eidx],
                                    wos[hf][:, eidx * 128:(eidx + 1) * 128],
                                    yqf[:, hf, :],
                                    start=(hf == 0), stop=(hf == NH - 1))

                        for h in range(NH):
                            hp, h2 = h // 2, h % 2
                            vt = vts[hp]
                            # entries: two k-tiles per sc pair-tile.
                            # Diagonal tiles keep separate (truncated) exps;
                            # below-diagonal pairs share one wide exp to
                            # amortize ScalarE access latency.
                            entries = [[(4 * qc, 0, True),
                                        (4 * qc + 1, 128, True)],
                                       [(4 * qc + 2, 256, True),
                                        (4 * qc + 3, 384, True)]]
                            entries += [[(j, 0, False), (j + 1, 0, False)]
                                        for j in range(0, 4 * qc, 2)]
                            n = len(entries)
                            ypsum = psy.tile([128, 512], F32, name="ypsum",
                                             tag="yp")
                            acc = pac.tile([128, 512], BF, name="acc",
                                           tag="acc")
                            ats = {}

                            def emit(ei, h=h, qc=qc, entries=entries,
                                     acc=acc, ats=ats):
                                slots = entries[ei]
                                diag = slots[0][2]
                                sc = pss.tile([128, 2, 512], F32,
                                              name="sc", tag="sc")
                                for s, (jg, c0, _) in enumerate(slots):
                                    nc.tensor.matmul(
                                        sc[:, s, c0:512],
                                        kT[:, h, jg * 128:(jg + 1) * 128],
                                        qT[:, h,
                                           qc * 512 + c0:(qc + 1) * 512],
                                        start=True, stop=True)
                                at = pba.tile([128, 2, 512], BF,
                                              name="at", tag="at")
                                if not diag:
                                    nc.scalar.activation(
                                        out=at, in_=sc, func=EXP,
                                        scale=ALPHA)
                                else:
                                    for s, (jg, c0, _) in enumerate(slots):
                                        nc.scalar.activation(
                                            out=at[:, s, c0:512],
                                            in_=sc[:, s, c0:512],
                                            func=EXP, scale=ALPHA)
                                        nc.vector.tensor_mul(
                                            at[:, s, c0:c0 + 128],
                                            at[:, s, c0:c0 + 128], mtri)
                                if ei == 0:
                                    nc.vector.tensor_copy(out=acc,
                                                          in_=at[:, 0, :])
                                    s, (jg, c0, _) = 1, slots[1]
                                    nc.vector.tensor_add(
                                        acc[:, c0:512], acc[:, c0:512],
                                        at[:, 1, c0:512])
                                else:
                                    for s, (jg, c0, _) in enumerate(slots):
                                        nc.vector.tensor_add(
                                            acc[:, c0:512],
                                            acc[:, c0:512],
                                            at[:, s, c0:512])
                                ats[ei] = (at, slots)

                            for ei in range(min(la, n)):
                                emit(ei)
                            cinc()
                            for ei in range(n):
                                if ei + la < n:
                                    emit(ei + la)
                                at, slots = ats.pop(ei)
                                for s, (jg, c0, _) in enumerate(slots):
                                    nc.tensor.matmul(
                                        ypsum[:, c0:512],
                                        vt[:, jg,
                                           h2 * 128:(h2 + 1) * 128],
                                        at[:, s, c0:512],
                                        start=(ei == 0 and s == 0),
                                        stop=(ei == n - 1 and
                                              s == len(slots) - 1))
                                c_step(cpace)
                            pending[0] = (ypsum, acc, h, yq, qc)
                        cinc()
                        if cops is not None:
                            for eidx in range(2):
                                ot = pco.tile([128, 512], BF, name="ot",
                                              tag="ot")
                                nc.scalar.copy(out=ot, in_=cops[eidx])
                                nc.sync.dma_start(
                                    out=outT[eidx * 128:(eidx + 1) * 128,
                                             0:512],
                                    in_=ot)
                        c_step(200)  # drain any leftover out-proj steps
                        es = 2 if qc == 0 else 0
                        cg[0] = c_gen(yq, qc, es)
                        credit[0] = 0.0
                        nxt = 8 * (2 + 2 * (qc + 1))
                        reserve = 0 if qc == QC - 2 else 0
                        cpace = ((DT - es) * (NH + 1) - reserve) / nxt
                    c_step(200)

            pbr.release()
            pac.release()
            pba.release()
            pbv.release()
    nc.compile()
    return nc


_NC_CACHE = None


def _get_program():
    global _NC_CACHE
    if _NC_CACHE is None:
        _NC_CACHE = build_program()
    return _NC_CACHE


def _host_inputs(x, w_qkv, w_o):
    inv = 1.0 / (ROPE_BASE ** (np.arange(0, HD, 2, dtype=np.float64) / HD))
    ang = np.arange(L, dtype=np.float64)[:, None] * inv[None, :]
    chalf = np.cos(ang).astype(BF16)                           # [L, 64]
    shalf = np.sin(ang).astype(BF16)
    p = np.arange(128)[:, None]
    f = np.arange(128)[None, :]
    mtri = (p <= f).astype(BF16)                               # [128, 128]
    ident = np.eye(128).astype(BF16)
    # per-head [even dims | odd dims] permutation for q/k rows
    perm = np.concatenate([np.arange(0, 128, 2), np.arange(1, 128, 2)])

    in_maps = []
    for c in range(8):
        b, g = c % 4, c // 4
        qr = w_qkv[g * DL:(g + 1) * DL]
        kr = w_qkv[D + g * DL:D + (g + 1) * DL]
        vr = w_qkv[2 * D + g * DL:2 * D + (g + 1) * DL]
        qp = qr.reshape(NH, HD, D)[:, perm, :].reshape(DL, D)
        kp = kr.reshape(NH, HD, D)[:, perm, :].reshape(DL, D)
        wqkvT = np.ascontiguousarray(
            np.concatenate([qp[:512], kp[:512], vr[:512],
                            vr[512:], qp[512:], kp[512:]], axis=0).T
        ).astype(BF16)
        in_maps.append({
            "xT": np.ascontiguousarray(x[b].T).astype(BF16),
            "wqkvT": wqkvT,
            "woT": np.ascontiguousarray(
                w_o[:, g * DL:(g + 1) * DL].T).astype(BF16),
            "chalf": chalf,
            "shalf": shalf,
            "mtri": mtri,
            "ident": ident,
        })
    return in_maps


def kernel(x, w_qkv, w_o, _trace=False):
    x = np.asarray(x, dtype=np.float32)
    w_qkv = np.asarray(w_qkv, dtype=np.float32)
    w_o = np.asarray(w_o, dtype=np.float32)
    nc = _get_program()
    in_maps = _host_inputs(x, w_qkv, w_o)
    res = run_bass_kernel_spmd(nc, in_maps, core_ids=list(range(8)),
                               trace=_trace)
    kernel.last_result = res
    parts = [np.asarray(r["outT"], dtype=np.float32) for r in res.results]
    out = np.empty((B, L, D), dtype=np.float32)
    for b in range(B):
        out[b] = (parts[b] + parts[b + 4]).T
    return out
